# revision 52
# baseline (speedup 1.0000x reference)
"""Causal self-attention (transformer block) on 8 trn2 NeuronCores.

Data-parallel over batch: core i processes batch element i (B=8).
Per-core dataflow (T=1024, C=768, H=12 heads, hd=64), all matmul
operands fp16 with fp32 PSUM accumulation:

  x [T,C] --PE transpose--> xT [C,T]            (feature-major)
  qkT[m]  = W_attn[:,m].T @ xT + b  [1536,T]    (feature-major q,k)
  v[t]    = xT[t].T @ W_attn[:,v] + b [T,768]   (row-major, +ones col)
  S^T[j,i] = sum_d kT[d,j] qT[d,i]              (scores transposed)
  E = exp(S^T*scale) * causal_mask              (masked -> exact 0)
  psY = sum_j [v_j | 1].T @ E_j                 (row 64 = softmax denom)
  yT = psY[0:64] * bcast(1/psY[64])             (feature-major y)
  out[t] = yT[:,t].T @ W_proj + b               (row-major, DMA out)

Schedule highlights (engine queues are FIFO; emission order is the
schedule):
- causal suffix trim: for diagonal key tiles only query columns
  >= 128*cd are touched (S matmul, exp, AV) -- the fully-masked prefix
  is never computed and E never needs zero-filling.
- AV is split into an unmasked-suffix matmul (fires right after exp)
  plus the 128-wide diagonal block (after the DVE mask multiply), so
  the mask is off the PE critical path.
- per-(head-pair) softmax normalization; the psb broadcast (1/den) and
  yT multiplies are emitted one pair late in group 0 so the psb matmul
  always has a pair of PE work between it and the DVE reciprocal.
- projection of token tiles 0..3 runs inside the group-1 pair loop;
  tiles 4..7 partial-accumulate k<=3/4 into f16 SBUF during pairs 4/5
  so only k=4..5 chunks and evacuations trail the final norm (keeps
  the PE HAM duty-cycle warm through the tail).
- W_attn is DMA'd in [768,128] f32 column slices (order 0,6,v,1,7,...)
  so the first qk GEMMs start as early as possible; fp16 casts of the
  slices are spread over ACT/DVE as fillers.
"""
import numpy as np

import concourse.bass as bass
import concourse.tile as tile
from concourse import bacc, mybir
from concourse.bass_utils import run_bass_kernel_spmd
from concourse.masks import make_identity

f32 = mybir.dt.float32
f32r = mybir.dt.float32r
f16 = mybir.dt.float16
Exp = mybir.ActivationFunctionType.Exp

B = 8
T = 1024
C = 768
H = 12
HD = 64
SCALE = HD ** -0.5
KC = C // 128        # 6 feature chunks
MT = T // 128        # 8 token tiles
GW = 512             # Tq group width
NG = T // GW         # 2 groups
VS = HD + 1          # per-head stride in v tile (ones column at position 64)
VW = H * VS          # v tile width incl. ones column (780)


def build_nc():
    nc = bacc.Bacc(None)
    x = nc.dram_tensor("x", [T, C], f32, kind="ExternalInput")
    W_attn = nc.dram_tensor("W_attn", [C, 3 * C], f32, kind="ExternalInput")
    b_attn = nc.dram_tensor("b_attn", [3 * C], f32, kind="ExternalInput")
    W_proj = nc.dram_tensor("W_proj", [C, C], f32, kind="ExternalInput")
    b_proj = nc.dram_tensor("b_proj", [C], f32, kind="ExternalInput")
    out = nc.dram_tensor("out", [T, C], f32, kind="ExternalOutput")

    with tile.TileContext(nc) as tc:
        with (
            tc.tile_pool(name="consts", bufs=1) as consts,
            tc.tile_pool(name="stage", bufs=4) as stage,
            tc.tile_pool(name="wstage", bufs=6) as wstage,
            tc.tile_pool(name="x16p", bufs=7) as x16p,
            tc.tile_pool(name="wq", bufs=1) as wq,
            tc.tile_pool(name="wp", bufs=1) as wp,
            tc.tile_pool(name="big", bufs=1) as big,
            tc.tile_pool(name="ep", bufs=3) as ep,
            tc.tile_pool(name="small", bufs=3) as small,
            tc.tile_pool(name="outp", bufs=2) as outp,
            tc.tile_pool(name="oacc", bufs=1) as oacc,
            tc.tile_pool(name="yup", bufs=4) as yup,
            tc.tile_pool(name="psG", bufs=2, space="PSUM") as psG,
            tc.tile_pool(name="psA", bufs=2, space="PSUM") as psA,
            tc.tile_pool(name="psY", bufs=2, space="PSUM") as psY,
        ):
            # ---- first x tile on the (idle) ACT hardware-DGE queue so its
            # descriptor issue overlaps the sync queue's own first issues
            x32s = {}
            x32_t0 = stage.tile([128, C], f32, tag="x32", name="x32_t0")
            nc.scalar.dma_start(out=x32_t0[:, :], in_=x[0:128, :])
            x32s[0] = x32_t0

            # ---- ACT table warm-up: the ~1.5us activation-table load runs
            # during the startup DMA waits, not before the first real ACT op
            warm16 = consts.tile([1, 16], f16, tag="warm16")
            nc.vector.memset(warm16[:, :], 1.0)
            nc.scalar.copy(warm16[0:1, 0:1], warm16[0:1, 8:9])

            # ---- gpsimd constants, then its queue is pure DMA issue ----
            ident = consts.tile([128, 128], f16, tag="ident")
            make_identity(nc, ident[:, :])

            # multiplicative causal mask for the E diagonal 128x128 block:
            # 1 where p<=f (valid), 0 elsewhere; applied to E after exp.
            mask01 = consts.tile([128, 128], f16, tag="mask01")
            nc.gpsimd.memset(mask01[:, :], 1.0)
            nc.gpsimd.affine_select(
                out=mask01[:, :], in_=mask01[:, :],
                compare_op=mybir.AluOpType.is_ge, fill=0.0,
                base=0, pattern=[[1, 128]], channel_multiplier=-1,
            )

            # ---- HAM warm-up: ~3us of dummy transposes while the first
            # DMAs are in flight. The PE clock gate defaults to half rate
            # and needs ~4us of sustained activity to open; burning the
            # otherwise-idle DMA-wait window here means the real startup
            # work runs at full clock. Results are never read.
            for i in range(28):
                dpool, dtag = (psA, "s") if (i % 2) else (psG, "g")
                dmy = dpool.tile([128, 128], f16, tag=dtag, name="dmy")
                nc.tensor.transpose(dmy[:, :], ident[:, :], ident[:, :])

            # ---- x f32 tiles (t=1..3) on the sync HWDGE queue ----
            def load_x(t):
                x32 = stage.tile([128, C], f32, tag="x32")
                nc.sync.dma_start(out=x32[:, :], in_=x[t * 128:(t + 1) * 128, :])
                x32s[t] = x32

            for t in range(1, 4):
                load_x(t)

            # ---- weights: f32 column slices on sync, cast on ACT/DVE ----
            Wqk = wq.tile([128, KC, 2 * C], f16, tag="Wqk", name="Wqk")
            Wv = wq.tile([128, KC, C], f16, tag="Wv", name="Wv")
            Wp16 = wp.tile([128, KC, C], f16, tag="Wp16", name="Wp16")

            x16s = {}
            w32s = {}

            def wdma(m):
                w32 = wstage.tile([128, KC, 128], f32, tag="w32")
                nc.sync.dma_start(
                    out=w32[:, :, :],
                    in_=W_attn[:, m * 128:(m + 1) * 128]
                    .rearrange("(k p) m -> p k m", p=128))
                w32s[m] = w32

            def cast_w(m, on_act):
                dst = (Wqk[:, :, m * 128:(m + 1) * 128] if m < 12
                       else Wv[:, :, (m - 12) * 128:(m - 11) * 128])
                if on_act:
                    nc.scalar.copy(dst, w32s[m][:, :, :])
                else:
                    nc.vector.tensor_copy(dst, w32s[m][:, :, :])

            def load_wp(k):
                wp32 = wstage.tile([128, C], f32, tag="wp32")
                nc.sync.dma_start(out=wp32[:, :], in_=W_proj[k * 128:(k + 1) * 128, :])
                nc.vector.tensor_copy(Wp16[:, k, :], wp32[:, :])

            wdma(0)
            wdma(6)

            # bias consts (small DMAs; <=3KB rows so descriptor gen is cheap)
            ones16 = consts.tile([1, 512], f16, tag="ones16")
            nc.vector.memset(ones16[:, :], 1.0)
            ones65 = consts.tile([65, 64], f16, tag="ones65")
            nc.vector.memset(ones65[:, :], 1.0)

            bcol = consts.tile([128, 3 * C // 128], f32, tag="bcol")
            nc.sync.dma_start(
                out=bcol[:, :],
                in_=b_attn.ap().rearrange("(m p) -> p m", p=128))
            bqv32 = consts.tile([1, C], f32, tag="bqv32")
            nc.sync.dma_start(
                out=bqv32[:, :],
                in_=b_attn[2 * C:3 * C].rearrange("(a d) -> a d", a=1))
            bp32 = consts.tile([1, C], f32, tag="bp32")
            nc.sync.dma_start(out=bp32[:, :], in_=b_proj.ap().rearrange("(a d) -> a d", a=1))

            for m in (12, 13, 14, 15, 16, 17, 1, 7):
                wdma(m)
            for t in range(4, MT):
                load_x(t)
            for m in (2, 8, 3, 9, 4, 10, 5, 11):
                wdma(m)

            # ---- x fp16 convert + transpose to xT [C, T] ----
            xT = [big.tile([128, T], f16, tag=f"xT_{c}", name=f"xT_{c}") for c in range(KC)]

            def emit_x16(t):
                x16 = x16p.tile([128, C], f16, tag="x16")
                # alternate ACT/DVE so neither engine serializes the startup
                if t % 2 == 0:
                    nc.scalar.copy(x16[:, :], x32s[t][:, :])
                else:
                    nc.vector.tensor_copy(x16[:, :], x32s[t][:, :])
                x16s[t] = x16

            tcount = [0]

            def emit_transpose(t):
                if t not in x16s:
                    emit_x16(t)
                x16 = x16s.pop(t)
                for c in range(KC):
                    i = tcount[0]
                    tcount[0] += 1
                    if t < 4:
                        # startup batch: alternate psum pool (psA is idle)
                        # and evacuation engine so PE never waits an evac
                        pool, tag = (psA, "s") if (i % 2) else (psG, "g")
                        pst = pool.tile([128, 128], f16, tag=tag, name="pst")
                        nc.tensor.transpose(
                            pst[:, :], x16[:, c * 128:(c + 1) * 128], ident[:, :])
                        if i % 2:
                            nc.scalar.copy(
                                xT[c][:, t * 128:(t + 1) * 128], pst[:, :])
                        else:
                            nc.vector.tensor_copy(
                                xT[c][:, t * 128:(t + 1) * 128], pst[:, :])
                    else:
                        # in-pair filler: psG only, DVE evac (ACT runs exp)
                        pst = psG.tile([128, 128], f16, tag="g", name="pst")
                        nc.tensor.transpose(
                            pst[:, :], x16[:, c * 128:(c + 1) * 128], ident[:, :])
                        nc.vector.tensor_copy(
                            xT[c][:, t * 128:(t + 1) * 128], pst[:, :])

            for t in range(4):
                emit_transpose(t)
            cast_w(0, True)
            cast_w(6, True)

            # ---- qk^T GEMM: qkT[m] [128, T] f16, m=0..11 covers features 0..1535
            qkT = [big.tile([128, T], f16, tag=f"qkT_{m}", name=f"qkT_{m}") for m in range(12)]

            def emit_qk(m, n, bias_dve=False):
                ps = psG.tile([128, 512], f32, tag="g", name="qk_ps")
                for k in range(KC):
                    nc.tensor.matmul(
                        ps[:, :], Wqk[:, k, m * 128:(m + 1) * 128],
                        xT[k][:, n * 512:(n + 1) * 512],
                        start=(k == 0), stop=(k == KC - 1))
                # bias is per-partition in the feature-major layout: fold it
                # into the psum->sbuf copy
                if n == 0 and not bias_dve:
                    nc.scalar.activation(
                        qkT[m][:, n * 512:(n + 1) * 512], ps[:, :],
                        mybir.ActivationFunctionType.Identity,
                        bias=bcol[:, m:m + 1])
                else:
                    nc.vector.tensor_scalar_add(
                        qkT[m][:, n * 512:(n + 1) * 512], ps[:, :],
                        bcol[:, m:m + 1])

            # ---- v rows: v_sb[t] [128, 780] f16 (64 cols + ones col per head)
            v_sb = [big.tile([128, VW], f16, tag=f"v_{t}", name=f"v_{t}") for t in range(MT)]

            def emit_v(t):
                vht = v_sb[t][:, :].rearrange("p (h s) -> p h s", s=VS)
                nc.vector.memset(vht[:, :, HD:HD + 1], 1.0)
                vchunks = ((0, 512), (512, 256))
                pss = [psG.tile([128, 512], f32, tag="g", name=f"v_ps{n}")
                       for n in range(2)]
                for k in range(KC):
                    for n, (c0, w) in enumerate(vchunks):
                        nc.tensor.matmul(
                            pss[n][:, 0:w], xT[k][:, t * 128:(t + 1) * 128],
                            Wv[:, k, c0:c0 + w],
                            start=(k == 0), stop=(k == KC - 1))
                for n, (c0, w) in enumerate(vchunks):
                    nh = w // HD
                    h0 = c0 // HD
                    nc.vector.tensor_add(
                        vht[:, h0:h0 + nh, 0:HD],
                        pss[n][:, 0:w].rearrange("p (h s) -> p h s", s=HD),
                        vbias[:, c0:c0 + w].rearrange("p (h s) -> p h s", s=HD))

            # ---- attention: S^T -> exp (-> mask) -> AV (+denom) -> normalize
            yT = [big.tile([128, T], f16, tag=f"yT_{m}", name=f"yT_{m}") for m in range(KC)]

            OCHUNKS = ((0, 512), (512, 256))
            proj_pss = {}

            def emit_proj_ks(t, ks):
                if t not in proj_pss:
                    proj_pss[t] = [psG.tile([128, 512], f32, tag="g",
                                            name=f"o_ps{n}") for n in range(2)]
                pss = proj_pss[t]
                for k in ks:
                    for n, (c0, w) in enumerate(OCHUNKS):
                        nc.tensor.matmul(
                            pss[n][:, 0:w], yT[k][:, t * 128:(t + 1) * 128],
                            Wp16[:, k, c0:c0 + w],
                            start=(k == 0), stop=(k == KC - 1))

            def emit_proj_fin(t):
                pss = proj_pss.pop(t)
                o_sb = outp.tile([128, C], f32, tag="o", name="o_sb")
                for n, (c0, w) in enumerate(OCHUNKS):
                    nc.vector.tensor_add(
                        o_sb[:, c0:c0 + w], pss[n][:, 0:w], pbias[:, c0:c0 + w])
                    nc.sync.dma_start(
                        out=out[t * 128:(t + 1) * 128, c0:c0 + w],
                        in_=o_sb[:, c0:c0 + w])

            # partial projection for the tail tiles: accumulate k chunks ks
            # into PSUM, then fold (+ pbias) into the f16 SBUF accumulator.
            o_acc = {t: oacc.tile([128, C], f16, tag=f"oacc_{t}", name=f"oacc_{t}")
                     for t in range(4, MT)}

            def emit_partial_ks(t, ks, k0, k1):
                # accumulate chunks ks of the k0..k1 partial group for tile t
                key = ("p", t)
                if key not in proj_pss:
                    proj_pss[key] = [psG.tile([128, 512], f32, tag="g",
                                              name=f"op_ps{n}") for n in range(2)]
                pss = proj_pss[key]
                for k in ks:
                    for n, (c0, w) in enumerate(OCHUNKS):
                        nc.tensor.matmul(
                            pss[n][:, 0:w], yT[k][:, t * 128:(t + 1) * 128],
                            Wp16[:, k, c0:c0 + w],
                            start=(k == k0), stop=(k == k1))

            def emit_partial_evac(t):
                pss = proj_pss.pop(("p", t))
                for n, (c0, w) in enumerate(OCHUNKS):
                    nc.vector.tensor_add(
                        o_acc[t][:, c0:c0 + w], pss[n][:, 0:w], pbias[:, c0:c0 + w])

            def emit_proj_partial(t, ks):
                emit_partial_ks(t, ks, ks[0], ks[-1])
                emit_partial_evac(t)

            def emit_proj_final(t, ks):
                pss = [psG.tile([128, 512], f32, tag="g", name=f"of_ps{n}")
                       for n in range(2)]
                for k in ks:
                    for n, (c0, w) in enumerate(OCHUNKS):
                        nc.tensor.matmul(
                            pss[n][:, 0:w], yT[k][:, t * 128:(t + 1) * 128],
                            Wp16[:, k, c0:c0 + w],
                            start=(k == ks[0]), stop=(k == ks[-1]))
                o_sb = outp.tile([128, C], f32, tag="o", name="o_sb")
                for n, (c0, w) in enumerate(OCHUNKS):
                    nc.vector.tensor_add(
                        o_sb[:, c0:c0 + w], pss[n][:, 0:w], o_acc[t][:, c0:c0 + w])
                    nc.sync.dma_start(
                        out=out[t * 128:(t + 1) * 128, c0:c0 + w],
                        in_=o_sb[:, c0:c0 + w])

            rec1s = {}

            def emit_pair(g, pr, filler=None, jfillers=None):
                hA, hB = 2 * pr, 2 * pr + 1
                qt = pr
                nchunks = 4 * g + 4
                psyA = psY.tile([65, GW], f32, tag="y", name="psyA")
                psyB = psY.tile([65, GW], f32, tag="y", name="psyB")
                for j in range(nchunks):
                    cd = j - 4 * g  # diagonal col-block index
                    c0 = max(cd, 0) * 128  # masked-prefix width: skip it
                    psS = psA.tile([128, 2, GW], f32, tag="s", name="psS")
                    nc.tensor.matmul(
                        psS[:, 0, c0:GW],
                        qkT[6 + qt][0:64, j * 128:(j + 1) * 128],
                        qkT[qt][0:64, g * GW + c0:(g + 1) * GW],
                        start=True, stop=True)
                    nc.tensor.matmul(
                        psS[:, 1, c0:GW],
                        qkT[6 + qt][64:128, j * 128:(j + 1) * 128],
                        qkT[qt][64:128, g * GW + c0:(g + 1) * GW],
                        start=True, stop=True)
                    E2 = ep.tile([128, 2, GW], f16, tag="e", name="E2")
                    nc.scalar.activation(
                        E2[:, :, c0:GW], psS[:, :, c0:GW], Exp, scale=SCALE)
                    if cd >= 0:
                        # zero the strictly-upper triangle of the diagonal
                        # block (DVE); the unmasked AV suffix below does not
                        # wait for it.
                        nc.vector.tensor_mul(
                            E2[:, 0:2, c0:c0 + 128], E2[:, 0:2, c0:c0 + 128],
                            mask01[:, :].unsqueeze(1).broadcast_to((128, 2, 128)))
                    if j == 0 and filler is not None:
                        # PE filler between the first S pair and its AV (the
                        # AV waits on the exp latency at pair startup).
                        filler()
                    if jfillers is not None and j in jfillers:
                        jfillers[j]()
                    last = j == nchunks - 1
                    cm = c0 + 128  # end of the diagonal (masked) block
                    for ei, (h, psy) in enumerate(((hA, psyA), (hB, psyB))):
                        vsl = v_sb[j][:, h * VS:h * VS + HD + 1]
                        if cd >= 0:
                            if cm < GW:
                                # unmasked suffix: no mask dependency
                                nc.tensor.matmul(
                                    psy[:, cm:GW], vsl, E2[:, ei, cm:GW],
                                    start=(j == 0), stop=False,
                                    skip_group_check=True)
                            # masked diagonal block (waits on the DVE mask)
                            nc.tensor.matmul(
                                psy[:, c0:cm], vsl, E2[:, ei, c0:cm],
                                start=False, stop=last,
                                skip_group_check=True)
                        else:
                            nc.tensor.matmul(
                                psy[:, :], vsl, E2[:, ei, :],
                                start=(j == 0), stop=last,
                                skip_group_check=True)
                # readout: yu + den per head (psY recycles as early as
                # possible), then the shared reciprocal chain. For the LAST
                # pair there is no next pair waiting on psY, so the
                # denominators go first and the tail psb matmul waits ~1.3us
                # less.
                yus = {}
                den2 = small.tile([33, GW], f32, tag="den2", name=f"den2_{g}_{pr}")
                den_first = (g, pr) == (1, 5)
                if den_first:
                    # whole reciprocal chain ahead of the y copies in the
                    # DVE FIFO: the tail psb matmul waits ~1.3us less
                    nc.vector.tensor_copy(den2[0:1, :], psyA[64:65, :])
                    nc.vector.tensor_copy(den2[32:33, :], psyB[64:65, :])
                else:
                    for i, (h, psy) in enumerate(((hA, psyA), (hB, psyB))):
                        yu = yup.tile([64, GW], f16, tag="yu", name="yu")
                        nc.vector.tensor_copy(yu[:, :], psy[0:64, :])
                        nc.vector.tensor_copy(den2[32 * i:32 * i + 1, :], psy[64:65, :])
                        yus[h] = yu
                rec2 = small.tile([33, GW], f32, tag="rec2", name=f"rec2_{g}_{pr}")
                nc.vector.reciprocal_approx_fast(out=rec2[:, :], in_=den2[:, :])
                rec16 = small.tile([33, GW], f16, tag="rec16", name=f"rec16_{g}_{pr}")
                nc.vector.tensor_copy(rec16[:, :], rec2[:, :])
                rec1s[(g, 2 * pr)] = rec16
                rec1s[(g, 2 * pr + 1)] = rec16
                if den_first:
                    for i, (h, psy) in enumerate(((hA, psyA), (hB, psyB))):
                        yu = yup.tile([64, GW], f16, tag="yu", name="yu")
                        nc.vector.tensor_copy(yu[:, :], psy[0:64, :])
                        yus[h] = yu
                return yus

            def emit_norm_apply(g, pr, yus):
                for i in range(2):
                    h = 2 * pr + i
                    qt, qp = h // 2, (h % 2) * 64
                    psb = psG.tile([64, GW], f32, tag="g", name="psb")
                    nc.tensor.matmul(
                        psb[:, :], ones65[32 * i:32 * i + 1, :],
                        rec1s[(g, h)][32 * i:32 * i + 1, :], start=True, stop=True)
                    nc.vector.tensor_mul(
                        yT[qt][qp:qp + 64, g * GW:(g + 1) * GW],
                        yus[h][:, :], psb[:, :])

            # ---- group-0 prelude: first qk GEMMs, bias tiles, v tiles ----
            yus_all = {0: {}, 1: {}}
            emit_qk(0, 0)
            emit_qk(6, 0)

            # bias f16 casts + broadcast tiles (bqv32/bp32 landed long ago)
            bqv16 = consts.tile([1, C], f16, tag="bqv16")
            nc.vector.tensor_copy(bqv16[:, :], bqv32[:, :])
            bp16 = consts.tile([1, C], f16, tag="bp16")
            nc.vector.tensor_copy(bp16[:, :], bp32[:, :])
            vb_ps = psG.tile([128, 512], f32, tag="g", name="vb_ps")
            pbias = consts.tile([128, C], f32, tag="pbias")
            vbias = consts.tile([128, C], f16, tag="vbias")
            for c0 in (0, 512):
                w = min(512, C - c0)
                nc.tensor.matmul(
                    vb_ps[:, 0:w], ones16[0:1, 0:128],
                    bqv16[0:1, c0:c0 + w], start=True, stop=True)
                nc.vector.tensor_copy(vbias[:, c0:c0 + w], vb_ps[:, 0:w])
            for c0 in (0, 512):
                w = min(512, C - c0)
                pb_ps = psG.tile([128, 512], f32, tag="g", name="pb_ps")
                nc.tensor.matmul(
                    pb_ps[:, 0:w], ones16[0:1, 0:128],
                    bp16[0:1, c0:c0 + w], start=True, stop=True)
                nc.vector.tensor_copy(pbias[:, c0:c0 + w], pb_ps[:, 0:w])

            for mi, mv in enumerate(range(12, 18)):
                cast_w(mv, mi % 2 == 0)
            for t in range(4, MT):
                emit_x16(t)
            for t in range(4):
                emit_v(t)
            cast_w(1, True)
            cast_w(7, True)
            emit_qk(1, 0)
            emit_qk(7, 0)

            # ---- group-0 pair loop ----
            fill_cast = {0: (2, 8), 1: (3, 9), 2: (4, 10), 3: (5, 11)}
            fill_qk0 = {1: (2, 8), 2: (3, 9), 3: (4, 10), 4: (5, 11)}
            fill_v = {2: 4, 3: 5, 4: 6, 5: 7}
            fill_wp = {2: (0, 1, 2), 3: (3, 4, 5)}
            g0_fillers = {
                0: (lambda: emit_transpose(4)),
                1: (lambda: emit_transpose(5)),
                2: (lambda: emit_transpose(6)),
                3: (lambda: emit_qk(1, 1)),
                4: (lambda: emit_qk(2, 1)),
                5: (lambda: emit_qk(3, 1)),
            }
            fill_qk1post = {2: (0, 6), 3: (7,), 4: (8,), 5: (9,)}
            for pr in range(6):
                yus_all[0].update(emit_pair(0, pr, filler=g0_fillers[pr]))
                if pr == 2:
                    emit_transpose(7)
                for mi, m in enumerate(fill_cast.get(pr, ())):
                    cast_w(m, mi % 2 == 0)
                for m in fill_qk0.get(pr, ()):
                    emit_qk(m, 0, bias_dve=True)
                for m in fill_qk1post.get(pr, ()):
                    emit_qk(m, 1)
                if pr in fill_v:
                    emit_v(fill_v[pr])
                for k in fill_wp.get(pr, ()):
                    load_wp(k)
                # delay each norm apply by one pair: its psb matmul waits on
                # the DVE reciprocal, so give it PE-filler lead time.
                if pr >= 1:
                    emit_norm_apply(0, pr - 1, yus_all[0])
            for m in (4, 10, 5, 11):
                emit_qk(m, 1)
            emit_norm_apply(0, 5, yus_all[0])

            # ---- group-1 pair loop ----
            for pr in range(6):
                if pr < 4:
                    filler = lambda t=pr: emit_proj_ks(t, [0, 1, 2])
                elif pr == 4:
                    filler = lambda: emit_proj_partial(5, [0, 1, 2, 3])
                else:
                    filler = lambda: emit_proj_partial(6, [0, 1, 2, 3, 4])
                yus_all[1].update(emit_pair(1, pr, filler=filler))
                if pr < 4:
                    emit_proj_ks(pr, [3, 4, 5])
                    emit_proj_fin(pr)
                    emit_norm_apply(1, pr, yus_all[1])
                elif pr == 4:
                    emit_proj_partial(4, [0, 1, 2, 3])
                    emit_norm_apply(1, 4, yus_all[1])
            emit_proj_partial(7, [0, 1, 2, 3, 4])
            emit_norm_apply(1, 5, yus_all[1])
            emit_proj_final(4, [4, 5])
            emit_proj_final(5, [4, 5])
            emit_proj_final(6, [5])
            emit_proj_final(7, [5])

    nc.finalize()
    return nc


_CACHE = {}


def _get_nc():
    if "nc" not in _CACHE:
        _CACHE["nc"] = build_nc()
    return _CACHE["nc"]


def run(inputs, trace=False):
    nc = _get_nc()
    x = np.asarray(inputs["x"], dtype=np.float32)
    in_maps = [
        {
            "x": np.ascontiguousarray(x[i]),
            "W_attn": np.asarray(inputs["W_attn"], dtype=np.float32),
            "b_attn": np.asarray(inputs["b_attn"], dtype=np.float32),
            "W_proj": np.asarray(inputs["W_proj"], dtype=np.float32),
            "b_proj": np.asarray(inputs["b_proj"], dtype=np.float32),
        }
        for i in range(B)
    ]
    res = run_bass_kernel_spmd(nc, in_maps, core_ids=list(range(B)), trace=trace)
    y = np.stack([res.results[i]["out"] for i in range(B)], axis=0)
    return y, res


def kernel(**inputs):
    y, _ = run(inputs, trace=False)
    return y


# revision 53
# speedup vs baseline: 1.0024x; 1.0024x over previous
"""Causal self-attention (transformer block) on 8 trn2 NeuronCores.

Data-parallel over batch: core i processes batch element i (B=8).
Per-core dataflow (T=1024, C=768, H=12 heads, hd=64), all matmul
operands fp16 with fp32 PSUM accumulation:

  x [T,C] --PE transpose--> xT [C,T]            (feature-major)
  qkT[m]  = W_attn[:,m].T @ xT + b  [1536,T]    (feature-major q,k)
  v[t]    = xT[t].T @ W_attn[:,v] + b [T,768]   (row-major, +ones col)
  S^T[j,i] = sum_d kT[d,j] qT[d,i]              (scores transposed)
  E = exp(S^T*scale) * causal_mask              (masked -> exact 0)
  psY = sum_j [v_j | 1].T @ E_j                 (row 64 = softmax denom)
  yT = psY[0:64] * bcast(1/psY[64])             (feature-major y)
  out[t] = yT[:,t].T @ W_proj + b               (row-major, DMA out)

Schedule highlights (engine queues are FIFO; emission order is the
schedule):
- causal suffix trim: for diagonal key tiles only query columns
  >= 128*cd are touched (S matmul, exp, AV) -- the fully-masked prefix
  is never computed and E never needs zero-filling.
- AV is split into an unmasked-suffix matmul (fires right after exp)
  plus the 128-wide diagonal block (after the DVE mask multiply), so
  the mask is off the PE critical path.
- per-(head-pair) softmax normalization; the psb broadcast (1/den) and
  yT multiplies are emitted one pair late in group 0 so the psb matmul
  always has a pair of PE work between it and the DVE reciprocal.
- projection of token tiles 0..3 runs inside the group-1 pair loop;
  tiles 4..7 partial-accumulate k<=3/4 into f16 SBUF during pairs 4/5
  so only k=4..5 chunks and evacuations trail the final norm (keeps
  the PE HAM duty-cycle warm through the tail).
- W_attn is DMA'd in [768,128] f32 column slices (order 0,6,v,1,7,...)
  so the first qk GEMMs start as early as possible; fp16 casts of the
  slices are spread over ACT/DVE as fillers.
"""
import numpy as np

import concourse.bass as bass
import concourse.tile as tile
from concourse import bacc, mybir
from concourse.bass_utils import run_bass_kernel_spmd
from concourse.masks import make_identity

f32 = mybir.dt.float32
f32r = mybir.dt.float32r
f16 = mybir.dt.float16
Exp = mybir.ActivationFunctionType.Exp

B = 8
T = 1024
C = 768
H = 12
HD = 64
SCALE = HD ** -0.5
KC = C // 128        # 6 feature chunks
MT = T // 128        # 8 token tiles
GW = 512             # Tq group width
NG = T // GW         # 2 groups
VS = HD + 1          # per-head stride in v tile (ones column at position 64)
VW = H * VS          # v tile width incl. ones column (780)


def build_nc():
    nc = bacc.Bacc(None)
    x = nc.dram_tensor("x", [T, C], f32, kind="ExternalInput")
    W_attn = nc.dram_tensor("W_attn", [C, 3 * C], f32, kind="ExternalInput")
    b_attn = nc.dram_tensor("b_attn", [3 * C], f32, kind="ExternalInput")
    W_proj = nc.dram_tensor("W_proj", [C, C], f32, kind="ExternalInput")
    b_proj = nc.dram_tensor("b_proj", [C], f32, kind="ExternalInput")
    out = nc.dram_tensor("out", [T, C], f32, kind="ExternalOutput")

    with tile.TileContext(nc) as tc:
        with (
            tc.tile_pool(name="consts", bufs=1) as consts,
            tc.tile_pool(name="stage", bufs=4) as stage,
            tc.tile_pool(name="wstage", bufs=6) as wstage,
            tc.tile_pool(name="x16p", bufs=7) as x16p,
            tc.tile_pool(name="wq", bufs=1) as wq,
            tc.tile_pool(name="wp", bufs=1) as wp,
            tc.tile_pool(name="big", bufs=1) as big,
            tc.tile_pool(name="ep", bufs=3) as ep,
            tc.tile_pool(name="small", bufs=3) as small,
            tc.tile_pool(name="outp", bufs=2) as outp,
            tc.tile_pool(name="oacc", bufs=1) as oacc,
            tc.tile_pool(name="yup", bufs=4) as yup,
            tc.tile_pool(name="psG", bufs=2, space="PSUM") as psG,
            tc.tile_pool(name="psA", bufs=2, space="PSUM") as psA,
            tc.tile_pool(name="psY", bufs=2, space="PSUM") as psY,
        ):
            # ---- first x tile on the (idle) ACT hardware-DGE queue so its
            # descriptor issue overlaps the sync queue's own first issues
            x32s = {}
            x32_t0 = stage.tile([128, C], f32, tag="x32", name="x32_t0")
            nc.scalar.dma_start(out=x32_t0[:, :], in_=x[0:128, :])
            x32s[0] = x32_t0

            # ---- ACT table warm-up: the ~1.5us activation-table load runs
            # during the startup DMA waits, not before the first real ACT op
            warm16 = consts.tile([1, 16], f16, tag="warm16")
            nc.vector.memset(warm16[:, :], 1.0)
            nc.scalar.copy(warm16[0:1, 0:1], warm16[0:1, 8:9])

            # ---- gpsimd constants, then its queue is pure DMA issue ----
            ident = consts.tile([128, 128], f16, tag="ident")
            make_identity(nc, ident[:, :])

            # multiplicative causal mask for the E diagonal 128x128 block:
            # 1 where p<=f (valid), 0 elsewhere; applied to E after exp.
            mask01 = consts.tile([128, 128], f16, tag="mask01")
            nc.gpsimd.memset(mask01[:, :], 1.0)
            nc.gpsimd.affine_select(
                out=mask01[:, :], in_=mask01[:, :],
                compare_op=mybir.AluOpType.is_ge, fill=0.0,
                base=0, pattern=[[1, 128]], channel_multiplier=-1,
            )

            # ---- x f32 tiles (t=1..3) on the sync HWDGE queue ----
            def load_x(t):
                x32 = stage.tile([128, C], f32, tag="x32")
                nc.sync.dma_start(out=x32[:, :], in_=x[t * 128:(t + 1) * 128, :])
                x32s[t] = x32

            for t in range(1, 4):
                load_x(t)

            # ---- weights: f32 column slices on sync, cast on ACT/DVE ----
            Wqk = wq.tile([128, KC, 2 * C], f16, tag="Wqk", name="Wqk")
            Wv = wq.tile([128, KC, C], f16, tag="Wv", name="Wv")
            Wp16 = wp.tile([128, KC, C], f16, tag="Wp16", name="Wp16")

            x16s = {}
            w32s = {}

            def wdma(m):
                w32 = wstage.tile([128, KC, 128], f32, tag="w32")
                nc.sync.dma_start(
                    out=w32[:, :, :],
                    in_=W_attn[:, m * 128:(m + 1) * 128]
                    .rearrange("(k p) m -> p k m", p=128))
                w32s[m] = w32

            def cast_w(m, on_act):
                dst = (Wqk[:, :, m * 128:(m + 1) * 128] if m < 12
                       else Wv[:, :, (m - 12) * 128:(m - 11) * 128])
                if on_act:
                    nc.scalar.copy(dst, w32s[m][:, :, :])
                else:
                    nc.vector.tensor_copy(dst, w32s[m][:, :, :])

            def load_wp(k):
                wp32 = wstage.tile([128, C], f32, tag="wp32")
                nc.sync.dma_start(out=wp32[:, :], in_=W_proj[k * 128:(k + 1) * 128, :])
                nc.vector.tensor_copy(Wp16[:, k, :], wp32[:, :])

            wdma(0)
            wdma(6)

            # bias consts (small DMAs; <=3KB rows so descriptor gen is cheap)
            ones16 = consts.tile([1, 512], f16, tag="ones16")
            nc.vector.memset(ones16[:, :], 1.0)
            ones65 = consts.tile([65, 64], f16, tag="ones65")
            nc.vector.memset(ones65[:, :], 1.0)

            bcol = consts.tile([128, 3 * C // 128], f32, tag="bcol")
            nc.sync.dma_start(
                out=bcol[:, :],
                in_=b_attn.ap().rearrange("(m p) -> p m", p=128))
            bqv32 = consts.tile([1, C], f32, tag="bqv32")
            nc.sync.dma_start(
                out=bqv32[:, :],
                in_=b_attn[2 * C:3 * C].rearrange("(a d) -> a d", a=1))
            bp32 = consts.tile([1, C], f32, tag="bp32")
            nc.sync.dma_start(out=bp32[:, :], in_=b_proj.ap().rearrange("(a d) -> a d", a=1))

            for m in (12, 13, 14, 15, 16, 17, 1, 7):
                wdma(m)
            for t in range(4, MT):
                load_x(t)
            for m in (2, 8, 3, 9, 4, 10, 5, 11):
                wdma(m)

            # ---- x fp16 convert + transpose to xT [C, T] ----
            xT = [big.tile([128, T], f16, tag=f"xT_{c}", name=f"xT_{c}") for c in range(KC)]

            def emit_x16(t):
                x16 = x16p.tile([128, C], f16, tag="x16")
                # alternate ACT/DVE so neither engine serializes the startup
                if t % 2 == 0:
                    nc.scalar.copy(x16[:, :], x32s[t][:, :])
                else:
                    nc.vector.tensor_copy(x16[:, :], x32s[t][:, :])
                x16s[t] = x16

            tcount = [0]

            def emit_transpose(t):
                if t not in x16s:
                    emit_x16(t)
                x16 = x16s.pop(t)
                for c in range(KC):
                    i = tcount[0]
                    tcount[0] += 1
                    if t < 4:
                        # startup batch: alternate psum pool (psA is idle)
                        # and evacuation engine so PE never waits an evac
                        pool, tag = (psA, "s") if (i % 2) else (psG, "g")
                        pst = pool.tile([128, 128], f16, tag=tag, name="pst")
                        nc.tensor.transpose(
                            pst[:, :], x16[:, c * 128:(c + 1) * 128], ident[:, :])
                        if i % 2:
                            nc.scalar.copy(
                                xT[c][:, t * 128:(t + 1) * 128], pst[:, :])
                        else:
                            nc.vector.tensor_copy(
                                xT[c][:, t * 128:(t + 1) * 128], pst[:, :])
                    else:
                        # in-pair filler: psG only, DVE evac (ACT runs exp)
                        pst = psG.tile([128, 128], f16, tag="g", name="pst")
                        nc.tensor.transpose(
                            pst[:, :], x16[:, c * 128:(c + 1) * 128], ident[:, :])
                        nc.vector.tensor_copy(
                            xT[c][:, t * 128:(t + 1) * 128], pst[:, :])

            for t in range(4):
                emit_transpose(t)
            cast_w(0, True)
            cast_w(6, True)

            # ---- qk^T GEMM: qkT[m] [128, T] f16, m=0..11 covers features 0..1535
            qkT = [big.tile([128, T], f16, tag=f"qkT_{m}", name=f"qkT_{m}") for m in range(12)]

            def emit_qk(m, n, bias_dve=False):
                ps = psG.tile([128, 512], f32, tag="g", name="qk_ps")
                for k in range(KC):
                    nc.tensor.matmul(
                        ps[:, :], Wqk[:, k, m * 128:(m + 1) * 128],
                        xT[k][:, n * 512:(n + 1) * 512],
                        start=(k == 0), stop=(k == KC - 1))
                # bias is per-partition in the feature-major layout: fold it
                # into the psum->sbuf copy
                if n == 0 and not bias_dve:
                    nc.scalar.activation(
                        qkT[m][:, n * 512:(n + 1) * 512], ps[:, :],
                        mybir.ActivationFunctionType.Identity,
                        bias=bcol[:, m:m + 1])
                else:
                    nc.vector.tensor_scalar_add(
                        qkT[m][:, n * 512:(n + 1) * 512], ps[:, :],
                        bcol[:, m:m + 1])

            # ---- v rows: v_sb[t] [128, 780] f16 (64 cols + ones col per head)
            v_sb = [big.tile([128, VW], f16, tag=f"v_{t}", name=f"v_{t}") for t in range(MT)]

            def emit_v(t):
                vht = v_sb[t][:, :].rearrange("p (h s) -> p h s", s=VS)
                nc.vector.memset(vht[:, :, HD:HD + 1], 1.0)
                vchunks = ((0, 512), (512, 256))
                pss = [psG.tile([128, 512], f32, tag="g", name=f"v_ps{n}")
                       for n in range(2)]
                for k in range(KC):
                    for n, (c0, w) in enumerate(vchunks):
                        nc.tensor.matmul(
                            pss[n][:, 0:w], xT[k][:, t * 128:(t + 1) * 128],
                            Wv[:, k, c0:c0 + w],
                            start=(k == 0), stop=(k == KC - 1))
                for n, (c0, w) in enumerate(vchunks):
                    nh = w // HD
                    h0 = c0 // HD
                    nc.vector.tensor_add(
                        vht[:, h0:h0 + nh, 0:HD],
                        pss[n][:, 0:w].rearrange("p (h s) -> p h s", s=HD),
                        vbias[:, c0:c0 + w].rearrange("p (h s) -> p h s", s=HD))

            # ---- attention: S^T -> exp (-> mask) -> AV (+denom) -> normalize
            yT = [big.tile([128, T], f16, tag=f"yT_{m}", name=f"yT_{m}") for m in range(KC)]

            OCHUNKS = ((0, 512), (512, 256))
            proj_pss = {}

            def emit_proj_ks(t, ks):
                if t not in proj_pss:
                    proj_pss[t] = [psG.tile([128, 512], f32, tag="g",
                                            name=f"o_ps{n}") for n in range(2)]
                pss = proj_pss[t]
                for k in ks:
                    for n, (c0, w) in enumerate(OCHUNKS):
                        nc.tensor.matmul(
                            pss[n][:, 0:w], yT[k][:, t * 128:(t + 1) * 128],
                            Wp16[:, k, c0:c0 + w],
                            start=(k == 0), stop=(k == KC - 1))

            def emit_proj_fin(t):
                pss = proj_pss.pop(t)
                o_sb = outp.tile([128, C], f32, tag="o", name="o_sb")
                for n, (c0, w) in enumerate(OCHUNKS):
                    nc.vector.tensor_add(
                        o_sb[:, c0:c0 + w], pss[n][:, 0:w], pbias[:, c0:c0 + w])
                    nc.sync.dma_start(
                        out=out[t * 128:(t + 1) * 128, c0:c0 + w],
                        in_=o_sb[:, c0:c0 + w])

            # partial projection for the tail tiles: accumulate k chunks ks
            # into PSUM, then fold (+ pbias) into the f16 SBUF accumulator.
            o_acc = {t: oacc.tile([128, C], f16, tag=f"oacc_{t}", name=f"oacc_{t}")
                     for t in range(4, MT)}

            def emit_partial_ks(t, ks, k0, k1):
                # accumulate chunks ks of the k0..k1 partial group for tile t
                key = ("p", t)
                if key not in proj_pss:
                    proj_pss[key] = [psG.tile([128, 512], f32, tag="g",
                                              name=f"op_ps{n}") for n in range(2)]
                pss = proj_pss[key]
                for k in ks:
                    for n, (c0, w) in enumerate(OCHUNKS):
                        nc.tensor.matmul(
                            pss[n][:, 0:w], yT[k][:, t * 128:(t + 1) * 128],
                            Wp16[:, k, c0:c0 + w],
                            start=(k == k0), stop=(k == k1))

            def emit_partial_evac(t):
                pss = proj_pss.pop(("p", t))
                for n, (c0, w) in enumerate(OCHUNKS):
                    nc.vector.tensor_add(
                        o_acc[t][:, c0:c0 + w], pss[n][:, 0:w], pbias[:, c0:c0 + w])

            def emit_proj_partial(t, ks):
                emit_partial_ks(t, ks, ks[0], ks[-1])
                emit_partial_evac(t)

            def emit_proj_final(t, ks):
                pss = [psG.tile([128, 512], f32, tag="g", name=f"of_ps{n}")
                       for n in range(2)]
                for k in ks:
                    for n, (c0, w) in enumerate(OCHUNKS):
                        nc.tensor.matmul(
                            pss[n][:, 0:w], yT[k][:, t * 128:(t + 1) * 128],
                            Wp16[:, k, c0:c0 + w],
                            start=(k == ks[0]), stop=(k == ks[-1]))
                o_sb = outp.tile([128, C], f32, tag="o", name="o_sb")
                for n, (c0, w) in enumerate(OCHUNKS):
                    nc.vector.tensor_add(
                        o_sb[:, c0:c0 + w], pss[n][:, 0:w], o_acc[t][:, c0:c0 + w])
                    nc.sync.dma_start(
                        out=out[t * 128:(t + 1) * 128, c0:c0 + w],
                        in_=o_sb[:, c0:c0 + w])

            rec1s = {}

            def emit_pair(g, pr, filler=None, jfillers=None):
                hA, hB = 2 * pr, 2 * pr + 1
                qt = pr
                nchunks = 4 * g + 4
                psyA = psY.tile([65, GW], f32, tag="y", name="psyA")
                psyB = psY.tile([65, GW], f32, tag="y", name="psyB")
                for j in range(nchunks):
                    cd = j - 4 * g  # diagonal col-block index
                    c0 = max(cd, 0) * 128  # masked-prefix width: skip it
                    psS = psA.tile([128, 2, GW], f32, tag="s", name="psS")
                    nc.tensor.matmul(
                        psS[:, 0, c0:GW],
                        qkT[6 + qt][0:64, j * 128:(j + 1) * 128],
                        qkT[qt][0:64, g * GW + c0:(g + 1) * GW],
                        start=True, stop=True)
                    nc.tensor.matmul(
                        psS[:, 1, c0:GW],
                        qkT[6 + qt][64:128, j * 128:(j + 1) * 128],
                        qkT[qt][64:128, g * GW + c0:(g + 1) * GW],
                        start=True, stop=True)
                    E2 = ep.tile([128, 2, GW], f16, tag="e", name="E2")
                    nc.scalar.activation(
                        E2[:, :, c0:GW], psS[:, :, c0:GW], Exp, scale=SCALE)
                    if cd >= 0:
                        # zero the strictly-upper triangle of the diagonal
                        # block (DVE); the unmasked AV suffix below does not
                        # wait for it.
                        nc.vector.tensor_mul(
                            E2[:, 0:2, c0:c0 + 128], E2[:, 0:2, c0:c0 + 128],
                            mask01[:, :].unsqueeze(1).broadcast_to((128, 2, 128)))
                    if j == 0 and filler is not None:
                        # PE filler between the first S pair and its AV (the
                        # AV waits on the exp latency at pair startup).
                        filler()
                    if jfillers is not None and j in jfillers:
                        jfillers[j]()
                    last = j == nchunks - 1
                    cm = c0 + 128  # end of the diagonal (masked) block
                    for ei, (h, psy) in enumerate(((hA, psyA), (hB, psyB))):
                        vsl = v_sb[j][:, h * VS:h * VS + HD + 1]
                        if cd >= 0:
                            if cm < GW:
                                # unmasked suffix: no mask dependency
                                nc.tensor.matmul(
                                    psy[:, cm:GW], vsl, E2[:, ei, cm:GW],
                                    start=(j == 0), stop=False,
                                    skip_group_check=True)
                            # masked diagonal block (waits on the DVE mask)
                            nc.tensor.matmul(
                                psy[:, c0:cm], vsl, E2[:, ei, c0:cm],
                                start=False, stop=last,
                                skip_group_check=True)
                        else:
                            nc.tensor.matmul(
                                psy[:, :], vsl, E2[:, ei, :],
                                start=(j == 0), stop=last,
                                skip_group_check=True)
                # readout: yu + den per head (psY recycles as early as
                # possible), then the shared reciprocal chain. For the LAST
                # pair there is no next pair waiting on psY, so the
                # denominators go first and the tail psb matmul waits ~1.3us
                # less.
                yus = {}
                den2 = small.tile([33, GW], f32, tag="den2", name=f"den2_{g}_{pr}")
                den_first = (g, pr) == (1, 5)
                if den_first:
                    # whole reciprocal chain ahead of the y copies in the
                    # DVE FIFO: the tail psb matmul waits ~1.3us less
                    nc.vector.tensor_copy(den2[0:1, :], psyA[64:65, :])
                    nc.vector.tensor_copy(den2[32:33, :], psyB[64:65, :])
                else:
                    for i, (h, psy) in enumerate(((hA, psyA), (hB, psyB))):
                        yu = yup.tile([64, GW], f16, tag="yu", name="yu")
                        nc.vector.tensor_copy(yu[:, :], psy[0:64, :])
                        nc.vector.tensor_copy(den2[32 * i:32 * i + 1, :], psy[64:65, :])
                        yus[h] = yu
                rec2 = small.tile([33, GW], f32, tag="rec2", name=f"rec2_{g}_{pr}")
                nc.vector.reciprocal_approx_fast(out=rec2[:, :], in_=den2[:, :])
                rec16 = small.tile([33, GW], f16, tag="rec16", name=f"rec16_{g}_{pr}")
                nc.vector.tensor_copy(rec16[:, :], rec2[:, :])
                rec1s[(g, 2 * pr)] = rec16
                rec1s[(g, 2 * pr + 1)] = rec16
                if den_first:
                    for i, (h, psy) in enumerate(((hA, psyA), (hB, psyB))):
                        yu = yup.tile([64, GW], f16, tag="yu", name="yu")
                        nc.vector.tensor_copy(yu[:, :], psy[0:64, :])
                        yus[h] = yu
                return yus

            def emit_norm_apply(g, pr, yus):
                for i in range(2):
                    h = 2 * pr + i
                    qt, qp = h // 2, (h % 2) * 64
                    psb = psG.tile([64, GW], f32, tag="g", name="psb")
                    nc.tensor.matmul(
                        psb[:, :], ones65[32 * i:32 * i + 1, :],
                        rec1s[(g, h)][32 * i:32 * i + 1, :], start=True, stop=True)
                    nc.vector.tensor_mul(
                        yT[qt][qp:qp + 64, g * GW:(g + 1) * GW],
                        yus[h][:, :], psb[:, :])

            # ---- group-0 prelude: first qk GEMMs, bias tiles, v tiles ----
            yus_all = {0: {}, 1: {}}
            emit_qk(0, 0)
            emit_qk(6, 0)

            # bias f16 casts + broadcast tiles (bqv32/bp32 landed long ago)
            bqv16 = consts.tile([1, C], f16, tag="bqv16")
            nc.vector.tensor_copy(bqv16[:, :], bqv32[:, :])
            bp16 = consts.tile([1, C], f16, tag="bp16")
            nc.vector.tensor_copy(bp16[:, :], bp32[:, :])
            vb_ps = psG.tile([128, 512], f32, tag="g", name="vb_ps")
            pbias = consts.tile([128, C], f32, tag="pbias")
            vbias = consts.tile([128, C], f16, tag="vbias")
            for c0 in (0, 512):
                w = min(512, C - c0)
                nc.tensor.matmul(
                    vb_ps[:, 0:w], ones16[0:1, 0:128],
                    bqv16[0:1, c0:c0 + w], start=True, stop=True)
                nc.vector.tensor_copy(vbias[:, c0:c0 + w], vb_ps[:, 0:w])
            for c0 in (0, 512):
                w = min(512, C - c0)
                pb_ps = psG.tile([128, 512], f32, tag="g", name="pb_ps")
                nc.tensor.matmul(
                    pb_ps[:, 0:w], ones16[0:1, 0:128],
                    bp16[0:1, c0:c0 + w], start=True, stop=True)
                nc.vector.tensor_copy(pbias[:, c0:c0 + w], pb_ps[:, 0:w])

            for mi, mv in enumerate(range(12, 18)):
                cast_w(mv, mi % 2 == 0)
            for t in range(4, MT):
                emit_x16(t)
            for t in range(4):
                emit_v(t)
            cast_w(1, True)
            cast_w(7, True)
            emit_qk(1, 0)
            emit_qk(7, 0)

            # ---- group-0 pair loop ----
            fill_cast = {0: (2, 8), 1: (3, 9), 2: (4, 10), 3: (5, 11)}
            fill_qk0 = {1: (2, 8), 2: (3, 9), 3: (4, 10), 4: (5, 11)}
            fill_v = {2: 4, 3: 5, 4: 6, 5: 7}
            fill_wp = {2: (0, 1, 2), 3: (3, 4, 5)}
            g0_fillers = {
                0: (lambda: emit_transpose(4)),
                1: (lambda: emit_transpose(5)),
                2: (lambda: emit_transpose(6)),
                3: (lambda: emit_qk(1, 1)),
                4: (lambda: emit_qk(2, 1)),
                5: (lambda: emit_qk(3, 1)),
            }
            fill_qk1post = {2: (0, 6), 3: (7,), 4: (8,), 5: (9,)}
            for pr in range(6):
                yus_all[0].update(emit_pair(0, pr, filler=g0_fillers[pr]))
                if pr == 2:
                    emit_transpose(7)
                for mi, m in enumerate(fill_cast.get(pr, ())):
                    cast_w(m, mi % 2 == 0)
                for m in fill_qk0.get(pr, ()):
                    emit_qk(m, 0, bias_dve=True)
                for m in fill_qk1post.get(pr, ()):
                    emit_qk(m, 1)
                if pr in fill_v:
                    emit_v(fill_v[pr])
                for k in fill_wp.get(pr, ()):
                    load_wp(k)
                # delay each norm apply by one pair: its psb matmul waits on
                # the DVE reciprocal, so give it PE-filler lead time.
                if pr >= 1:
                    emit_norm_apply(0, pr - 1, yus_all[0])
            for m in (4, 10, 5, 11):
                emit_qk(m, 1)
            emit_norm_apply(0, 5, yus_all[0])

            # ---- group-1 pair loop ----
            for pr in range(6):
                if pr < 4:
                    filler = lambda t=pr: emit_proj_ks(t, [0, 1, 2])
                elif pr == 4:
                    filler = lambda: emit_proj_partial(5, [0, 1, 2, 3])
                else:
                    filler = lambda: emit_proj_partial(6, [0, 1, 2, 3, 4])
                yus_all[1].update(emit_pair(1, pr, filler=filler))
                if pr < 4:
                    emit_proj_ks(pr, [3, 4, 5])
                    emit_proj_fin(pr)
                    emit_norm_apply(1, pr, yus_all[1])
                elif pr == 4:
                    emit_proj_partial(4, [0, 1, 2, 3])
                    emit_norm_apply(1, 4, yus_all[1])
            emit_proj_partial(7, [0, 1, 2, 3, 4])
            emit_norm_apply(1, 5, yus_all[1])
            emit_proj_final(4, [4, 5])
            emit_proj_final(5, [4, 5])
            emit_proj_final(6, [5])
            emit_proj_final(7, [5])

    nc.finalize()
    return nc


_CACHE = {}


def _get_nc():
    if "nc" not in _CACHE:
        _CACHE["nc"] = build_nc()
    return _CACHE["nc"]


def run(inputs, trace=False):
    nc = _get_nc()
    x = np.asarray(inputs["x"], dtype=np.float32)
    in_maps = [
        {
            "x": np.ascontiguousarray(x[i]),
            "W_attn": np.asarray(inputs["W_attn"], dtype=np.float32),
            "b_attn": np.asarray(inputs["b_attn"], dtype=np.float32),
            "W_proj": np.asarray(inputs["W_proj"], dtype=np.float32),
            "b_proj": np.asarray(inputs["b_proj"], dtype=np.float32),
        }
        for i in range(B)
    ]
    res = run_bass_kernel_spmd(nc, in_maps, core_ids=list(range(B)), trace=trace)
    y = np.stack([res.results[i]["out"] for i in range(B)], axis=0)
    return y, res


def kernel(**inputs):
    y, _ = run(inputs, trace=False)
    return y


# revision 54
# speedup vs baseline: 1.0030x; 1.0006x over previous
"""Causal self-attention (transformer block) on 8 trn2 NeuronCores.

Data-parallel over batch: core i processes batch element i (B=8).
Per-core dataflow (T=1024, C=768, H=12 heads, hd=64), all matmul
operands fp16 with fp32 PSUM accumulation:

  x [T,C] --PE transpose--> xT [C,T]            (feature-major)
  qkT[m]  = W_attn[:,m].T @ xT + b  [1536,T]    (feature-major q,k)
  v[t]    = xT[t].T @ W_attn[:,v] + b [T,768]   (row-major, +ones col)
  S^T[j,i] = sum_d kT[d,j] qT[d,i]              (scores transposed)
  E = exp(S^T*scale) * causal_mask              (masked -> exact 0)
  psY = sum_j [v_j | 1].T @ E_j                 (row 64 = softmax denom)
  yT = psY[0:64] * bcast(1/psY[64])             (feature-major y)
  out[t] = yT[:,t].T @ W_proj + b               (row-major, DMA out)

Schedule highlights (engine queues are FIFO; emission order is the
schedule):
- causal suffix trim: for diagonal key tiles only query columns
  >= 128*cd are touched (S matmul, exp, AV) -- the fully-masked prefix
  is never computed and E never needs zero-filling.
- AV is split into an unmasked-suffix matmul (fires right after exp)
  plus the 128-wide diagonal block (after the DVE mask multiply), so
  the mask is off the PE critical path.
- per-(head-pair) softmax normalization; the psb broadcast (1/den) and
  yT multiplies are emitted one pair late in group 0 so the psb matmul
  always has a pair of PE work between it and the DVE reciprocal.
- projection of token tiles 0..3 runs inside the group-1 pair loop;
  tiles 4..7 partial-accumulate k<=3/4 into f16 SBUF during pairs 4/5
  so only k=4..5 chunks and evacuations trail the final norm (keeps
  the PE HAM duty-cycle warm through the tail).
- W_attn is DMA'd in [768,128] f32 column slices (order 0,6,v,1,7,...)
  so the first qk GEMMs start as early as possible; fp16 casts of the
  slices are spread over ACT/DVE as fillers.
"""
import numpy as np

import concourse.bass as bass
import concourse.tile as tile
from concourse import bacc, mybir
from concourse.bass_utils import run_bass_kernel_spmd
from concourse.masks import make_identity

f32 = mybir.dt.float32
f32r = mybir.dt.float32r
f16 = mybir.dt.float16
Exp = mybir.ActivationFunctionType.Exp

B = 8
T = 1024
C = 768
H = 12
HD = 64
SCALE = HD ** -0.5
KC = C // 128        # 6 feature chunks
MT = T // 128        # 8 token tiles
GW = 512             # Tq group width
NG = T // GW         # 2 groups
VS = HD + 1          # per-head stride in v tile (ones column at position 64)
VW = H * VS          # v tile width incl. ones column (780)


def build_nc():
    nc = bacc.Bacc(None)
    x = nc.dram_tensor("x", [T, C], f32, kind="ExternalInput")
    W_attn = nc.dram_tensor("W_attn", [C, 3 * C], f32, kind="ExternalInput")
    b_attn = nc.dram_tensor("b_attn", [3 * C], f32, kind="ExternalInput")
    W_proj = nc.dram_tensor("W_proj", [C, C], f32, kind="ExternalInput")
    b_proj = nc.dram_tensor("b_proj", [C], f32, kind="ExternalInput")
    out = nc.dram_tensor("out", [T, C], f32, kind="ExternalOutput")

    with tile.TileContext(nc) as tc:
        with (
            tc.tile_pool(name="consts", bufs=1) as consts,
            tc.tile_pool(name="stage", bufs=4) as stage,
            tc.tile_pool(name="wstage", bufs=6) as wstage,
            tc.tile_pool(name="x16p", bufs=7) as x16p,
            tc.tile_pool(name="wq", bufs=1) as wq,
            tc.tile_pool(name="wp", bufs=1) as wp,
            tc.tile_pool(name="big", bufs=1) as big,
            tc.tile_pool(name="ep", bufs=3) as ep,
            tc.tile_pool(name="small", bufs=3) as small,
            tc.tile_pool(name="outp", bufs=2) as outp,
            tc.tile_pool(name="oacc", bufs=1) as oacc,
            tc.tile_pool(name="yup", bufs=4) as yup,
            tc.tile_pool(name="psG", bufs=2, space="PSUM") as psG,
            tc.tile_pool(name="psA", bufs=2, space="PSUM") as psA,
            tc.tile_pool(name="psY", bufs=2, space="PSUM") as psY,
        ):
            # ---- first x tile on the (idle) ACT hardware-DGE queue so its
            # descriptor issue overlaps the sync queue's own first issues
            x32s = {}
            x32_t0 = stage.tile([128, C], f32, tag="x32", name="x32_t0")
            nc.scalar.dma_start(out=x32_t0[:, :], in_=x[0:128, :])
            x32s[0] = x32_t0

            # ---- ACT table warm-up: the ~1.5us activation-table load runs
            # during the startup DMA waits, not before the first real ACT op
            warm16 = consts.tile([1, 16], f16, tag="warm16")
            nc.vector.memset(warm16[:, :], 1.0)
            nc.scalar.copy(warm16[0:1, 0:1], warm16[0:1, 8:9])

            # ---- gpsimd constants, then its queue is pure DMA issue ----
            ident = consts.tile([128, 128], f16, tag="ident")
            make_identity(nc, ident[:, :])

            # multiplicative causal mask for the E diagonal 128x128 block:
            # 1 where p<=f (valid), 0 elsewhere; applied to E after exp.
            mask01 = consts.tile([128, 128], f16, tag="mask01")
            nc.gpsimd.memset(mask01[:, :], 1.0)
            nc.gpsimd.affine_select(
                out=mask01[:, :], in_=mask01[:, :],
                compare_op=mybir.AluOpType.is_ge, fill=0.0,
                base=0, pattern=[[1, 128]], channel_multiplier=-1,
            )

            # ---- HAM warm-up: ~4.8us of dummy transposes while the first
            # DMAs are in flight. The PE clock gate defaults to half rate
            # and opens after ~4us of sustained activity; burning the
            # otherwise-idle DMA-wait window here lets the real startup
            # work run at full clock. Results are never read, and the
            # stream drains into the arrival of the first x tile so the
            # activity window never resets.
            for i in range(44):
                dpool, dtag = (psA, "s") if (i % 2) else (psG, "g")
                dmy = dpool.tile([128, 128], f16, tag=dtag, name="dmy")
                nc.tensor.transpose(dmy[:, :], ident[:, :], ident[:, :])

            # ---- x f32 tiles (t=1..3) on the sync HWDGE queue ----
            def load_x(t):
                x32 = stage.tile([128, C], f32, tag="x32")
                nc.sync.dma_start(out=x32[:, :], in_=x[t * 128:(t + 1) * 128, :])
                x32s[t] = x32

            for t in range(1, 4):
                load_x(t)

            # ---- weights: f32 column slices on sync, cast on ACT/DVE ----
            Wqk = wq.tile([128, KC, 2 * C], f16, tag="Wqk", name="Wqk")
            Wv = wq.tile([128, KC, C], f16, tag="Wv", name="Wv")
            Wp16 = wp.tile([128, KC, C], f16, tag="Wp16", name="Wp16")

            x16s = {}
            w32s = {}

            def wdma(m):
                w32 = wstage.tile([128, KC, 128], f32, tag="w32")
                nc.sync.dma_start(
                    out=w32[:, :, :],
                    in_=W_attn[:, m * 128:(m + 1) * 128]
                    .rearrange("(k p) m -> p k m", p=128))
                w32s[m] = w32

            def cast_w(m, on_act):
                dst = (Wqk[:, :, m * 128:(m + 1) * 128] if m < 12
                       else Wv[:, :, (m - 12) * 128:(m - 11) * 128])
                if on_act:
                    nc.scalar.copy(dst, w32s[m][:, :, :])
                else:
                    nc.vector.tensor_copy(dst, w32s[m][:, :, :])

            def load_wp(k):
                wp32 = wstage.tile([128, C], f32, tag="wp32")
                nc.sync.dma_start(out=wp32[:, :], in_=W_proj[k * 128:(k + 1) * 128, :])
                nc.vector.tensor_copy(Wp16[:, k, :], wp32[:, :])

            wdma(0)
            wdma(6)

            # bias consts (small DMAs; <=3KB rows so descriptor gen is cheap)
            ones16 = consts.tile([1, 512], f16, tag="ones16")
            nc.vector.memset(ones16[:, :], 1.0)
            ones65 = consts.tile([65, 64], f16, tag="ones65")
            nc.vector.memset(ones65[:, :], 1.0)

            bcol = consts.tile([128, 3 * C // 128], f32, tag="bcol")
            nc.sync.dma_start(
                out=bcol[:, :],
                in_=b_attn.ap().rearrange("(m p) -> p m", p=128))
            bqv32 = consts.tile([1, C], f32, tag="bqv32")
            nc.sync.dma_start(
                out=bqv32[:, :],
                in_=b_attn[2 * C:3 * C].rearrange("(a d) -> a d", a=1))
            bp32 = consts.tile([1, C], f32, tag="bp32")
            nc.sync.dma_start(out=bp32[:, :], in_=b_proj.ap().rearrange("(a d) -> a d", a=1))

            for m in (12, 13, 14, 15, 16, 17, 1, 7):
                wdma(m)
            for t in range(4, MT):
                load_x(t)
            for m in (2, 8, 3, 9, 4, 10, 5, 11):
                wdma(m)

            # ---- x fp16 convert + transpose to xT [C, T] ----
            xT = [big.tile([128, T], f16, tag=f"xT_{c}", name=f"xT_{c}") for c in range(KC)]

            def emit_x16(t):
                x16 = x16p.tile([128, C], f16, tag="x16")
                # alternate ACT/DVE so neither engine serializes the startup
                if t % 2 == 0:
                    nc.scalar.copy(x16[:, :], x32s[t][:, :])
                else:
                    nc.vector.tensor_copy(x16[:, :], x32s[t][:, :])
                x16s[t] = x16

            tcount = [0]

            def emit_transpose(t):
                if t not in x16s:
                    emit_x16(t)
                x16 = x16s.pop(t)
                for c in range(KC):
                    i = tcount[0]
                    tcount[0] += 1
                    if t < 4:
                        # startup batch: alternate psum pool (psA is idle)
                        # and evacuation engine so PE never waits an evac
                        pool, tag = (psA, "s") if (i % 2) else (psG, "g")
                        pst = pool.tile([128, 128], f16, tag=tag, name="pst")
                        nc.tensor.transpose(
                            pst[:, :], x16[:, c * 128:(c + 1) * 128], ident[:, :])
                        if i % 2:
                            nc.scalar.copy(
                                xT[c][:, t * 128:(t + 1) * 128], pst[:, :])
                        else:
                            nc.vector.tensor_copy(
                                xT[c][:, t * 128:(t + 1) * 128], pst[:, :])
                    else:
                        # in-pair filler: psG only, DVE evac (ACT runs exp)
                        pst = psG.tile([128, 128], f16, tag="g", name="pst")
                        nc.tensor.transpose(
                            pst[:, :], x16[:, c * 128:(c + 1) * 128], ident[:, :])
                        nc.vector.tensor_copy(
                            xT[c][:, t * 128:(t + 1) * 128], pst[:, :])

            for t in range(4):
                emit_transpose(t)
            cast_w(0, True)
            cast_w(6, True)

            # ---- qk^T GEMM: qkT[m] [128, T] f16, m=0..11 covers features 0..1535
            qkT = [big.tile([128, T], f16, tag=f"qkT_{m}", name=f"qkT_{m}") for m in range(12)]

            def emit_qk(m, n, bias_dve=False):
                ps = psG.tile([128, 512], f32, tag="g", name="qk_ps")
                for k in range(KC):
                    nc.tensor.matmul(
                        ps[:, :], Wqk[:, k, m * 128:(m + 1) * 128],
                        xT[k][:, n * 512:(n + 1) * 512],
                        start=(k == 0), stop=(k == KC - 1))
                # bias is per-partition in the feature-major layout: fold it
                # into the psum->sbuf copy
                if n == 0 and not bias_dve:
                    nc.scalar.activation(
                        qkT[m][:, n * 512:(n + 1) * 512], ps[:, :],
                        mybir.ActivationFunctionType.Identity,
                        bias=bcol[:, m:m + 1])
                else:
                    nc.vector.tensor_scalar_add(
                        qkT[m][:, n * 512:(n + 1) * 512], ps[:, :],
                        bcol[:, m:m + 1])

            # ---- v rows: v_sb[t] [128, 780] f16 (64 cols + ones col per head)
            v_sb = [big.tile([128, VW], f16, tag=f"v_{t}", name=f"v_{t}") for t in range(MT)]

            def emit_v(t):
                vht = v_sb[t][:, :].rearrange("p (h s) -> p h s", s=VS)
                nc.vector.memset(vht[:, :, HD:HD + 1], 1.0)
                vchunks = ((0, 512), (512, 256))
                pss = [psG.tile([128, 512], f32, tag="g", name=f"v_ps{n}")
                       for n in range(2)]
                for k in range(KC):
                    for n, (c0, w) in enumerate(vchunks):
                        nc.tensor.matmul(
                            pss[n][:, 0:w], xT[k][:, t * 128:(t + 1) * 128],
                            Wv[:, k, c0:c0 + w],
                            start=(k == 0), stop=(k == KC - 1))
                for n, (c0, w) in enumerate(vchunks):
                    nh = w // HD
                    h0 = c0 // HD
                    nc.vector.tensor_add(
                        vht[:, h0:h0 + nh, 0:HD],
                        pss[n][:, 0:w].rearrange("p (h s) -> p h s", s=HD),
                        vbias[:, c0:c0 + w].rearrange("p (h s) -> p h s", s=HD))

            # ---- attention: S^T -> exp (-> mask) -> AV (+denom) -> normalize
            yT = [big.tile([128, T], f16, tag=f"yT_{m}", name=f"yT_{m}") for m in range(KC)]

            OCHUNKS = ((0, 512), (512, 256))
            proj_pss = {}

            def emit_proj_ks(t, ks):
                if t not in proj_pss:
                    proj_pss[t] = [psG.tile([128, 512], f32, tag="g",
                                            name=f"o_ps{n}") for n in range(2)]
                pss = proj_pss[t]
                for k in ks:
                    for n, (c0, w) in enumerate(OCHUNKS):
                        nc.tensor.matmul(
                            pss[n][:, 0:w], yT[k][:, t * 128:(t + 1) * 128],
                            Wp16[:, k, c0:c0 + w],
                            start=(k == 0), stop=(k == KC - 1))

            def emit_proj_fin(t):
                pss = proj_pss.pop(t)
                o_sb = outp.tile([128, C], f32, tag="o", name="o_sb")
                for n, (c0, w) in enumerate(OCHUNKS):
                    nc.vector.tensor_add(
                        o_sb[:, c0:c0 + w], pss[n][:, 0:w], pbias[:, c0:c0 + w])
                    nc.sync.dma_start(
                        out=out[t * 128:(t + 1) * 128, c0:c0 + w],
                        in_=o_sb[:, c0:c0 + w])

            # partial projection for the tail tiles: accumulate k chunks ks
            # into PSUM, then fold (+ pbias) into the f16 SBUF accumulator.
            o_acc = {t: oacc.tile([128, C], f16, tag=f"oacc_{t}", name=f"oacc_{t}")
                     for t in range(4, MT)}

            def emit_partial_ks(t, ks, k0, k1):
                # accumulate chunks ks of the k0..k1 partial group for tile t
                key = ("p", t)
                if key not in proj_pss:
                    proj_pss[key] = [psG.tile([128, 512], f32, tag="g",
                                              name=f"op_ps{n}") for n in range(2)]
                pss = proj_pss[key]
                for k in ks:
                    for n, (c0, w) in enumerate(OCHUNKS):
                        nc.tensor.matmul(
                            pss[n][:, 0:w], yT[k][:, t * 128:(t + 1) * 128],
                            Wp16[:, k, c0:c0 + w],
                            start=(k == k0), stop=(k == k1))

            def emit_partial_evac(t):
                pss = proj_pss.pop(("p", t))
                for n, (c0, w) in enumerate(OCHUNKS):
                    nc.vector.tensor_add(
                        o_acc[t][:, c0:c0 + w], pss[n][:, 0:w], pbias[:, c0:c0 + w])

            def emit_proj_partial(t, ks):
                emit_partial_ks(t, ks, ks[0], ks[-1])
                emit_partial_evac(t)

            def emit_proj_final(t, ks):
                pss = [psG.tile([128, 512], f32, tag="g", name=f"of_ps{n}")
                       for n in range(2)]
                for k in ks:
                    for n, (c0, w) in enumerate(OCHUNKS):
                        nc.tensor.matmul(
                            pss[n][:, 0:w], yT[k][:, t * 128:(t + 1) * 128],
                            Wp16[:, k, c0:c0 + w],
                            start=(k == ks[0]), stop=(k == ks[-1]))
                o_sb = outp.tile([128, C], f32, tag="o", name="o_sb")
                for n, (c0, w) in enumerate(OCHUNKS):
                    nc.vector.tensor_add(
                        o_sb[:, c0:c0 + w], pss[n][:, 0:w], o_acc[t][:, c0:c0 + w])
                    nc.sync.dma_start(
                        out=out[t * 128:(t + 1) * 128, c0:c0 + w],
                        in_=o_sb[:, c0:c0 + w])

            rec1s = {}

            def emit_pair(g, pr, filler=None, jfillers=None):
                hA, hB = 2 * pr, 2 * pr + 1
                qt = pr
                nchunks = 4 * g + 4
                psyA = psY.tile([65, GW], f32, tag="y", name="psyA")
                psyB = psY.tile([65, GW], f32, tag="y", name="psyB")
                for j in range(nchunks):
                    cd = j - 4 * g  # diagonal col-block index
                    c0 = max(cd, 0) * 128  # masked-prefix width: skip it
                    psS = psA.tile([128, 2, GW], f32, tag="s", name="psS")
                    nc.tensor.matmul(
                        psS[:, 0, c0:GW],
                        qkT[6 + qt][0:64, j * 128:(j + 1) * 128],
                        qkT[qt][0:64, g * GW + c0:(g + 1) * GW],
                        start=True, stop=True)
                    nc.tensor.matmul(
                        psS[:, 1, c0:GW],
                        qkT[6 + qt][64:128, j * 128:(j + 1) * 128],
                        qkT[qt][64:128, g * GW + c0:(g + 1) * GW],
                        start=True, stop=True)
                    E2 = ep.tile([128, 2, GW], f16, tag="e", name="E2")
                    nc.scalar.activation(
                        E2[:, :, c0:GW], psS[:, :, c0:GW], Exp, scale=SCALE)
                    if cd >= 0:
                        # zero the strictly-upper triangle of the diagonal
                        # block (DVE); the unmasked AV suffix below does not
                        # wait for it.
                        nc.vector.tensor_mul(
                            E2[:, 0:2, c0:c0 + 128], E2[:, 0:2, c0:c0 + 128],
                            mask01[:, :].unsqueeze(1).broadcast_to((128, 2, 128)))
                    if j == 0 and filler is not None:
                        # PE filler between the first S pair and its AV (the
                        # AV waits on the exp latency at pair startup).
                        filler()
                    if jfillers is not None and j in jfillers:
                        jfillers[j]()
                    last = j == nchunks - 1
                    cm = c0 + 128  # end of the diagonal (masked) block
                    for ei, (h, psy) in enumerate(((hA, psyA), (hB, psyB))):
                        vsl = v_sb[j][:, h * VS:h * VS + HD + 1]
                        if cd >= 0:
                            if cm < GW:
                                # unmasked suffix: no mask dependency
                                nc.tensor.matmul(
                                    psy[:, cm:GW], vsl, E2[:, ei, cm:GW],
                                    start=(j == 0), stop=False,
                                    skip_group_check=True)
                            # masked diagonal block (waits on the DVE mask)
                            nc.tensor.matmul(
                                psy[:, c0:cm], vsl, E2[:, ei, c0:cm],
                                start=False, stop=last,
                                skip_group_check=True)
                        else:
                            nc.tensor.matmul(
                                psy[:, :], vsl, E2[:, ei, :],
                                start=(j == 0), stop=last,
                                skip_group_check=True)
                # readout: yu + den per head (psY recycles as early as
                # possible), then the shared reciprocal chain. For the LAST
                # pair there is no next pair waiting on psY, so the
                # denominators go first and the tail psb matmul waits ~1.3us
                # less.
                yus = {}
                den2 = small.tile([33, GW], f32, tag="den2", name=f"den2_{g}_{pr}")
                den_first = (g, pr) == (1, 5)
                if den_first:
                    # whole reciprocal chain ahead of the y copies in the
                    # DVE FIFO: the tail psb matmul waits ~1.3us less
                    nc.vector.tensor_copy(den2[0:1, :], psyA[64:65, :])
                    nc.vector.tensor_copy(den2[32:33, :], psyB[64:65, :])
                else:
                    for i, (h, psy) in enumerate(((hA, psyA), (hB, psyB))):
                        yu = yup.tile([64, GW], f16, tag="yu", name="yu")
                        nc.vector.tensor_copy(yu[:, :], psy[0:64, :])
                        nc.vector.tensor_copy(den2[32 * i:32 * i + 1, :], psy[64:65, :])
                        yus[h] = yu
                rec2 = small.tile([33, GW], f32, tag="rec2", name=f"rec2_{g}_{pr}")
                nc.vector.reciprocal_approx_fast(out=rec2[:, :], in_=den2[:, :])
                rec16 = small.tile([33, GW], f16, tag="rec16", name=f"rec16_{g}_{pr}")
                nc.vector.tensor_copy(rec16[:, :], rec2[:, :])
                rec1s[(g, 2 * pr)] = rec16
                rec1s[(g, 2 * pr + 1)] = rec16
                if den_first:
                    for i, (h, psy) in enumerate(((hA, psyA), (hB, psyB))):
                        yu = yup.tile([64, GW], f16, tag="yu", name="yu")
                        nc.vector.tensor_copy(yu[:, :], psy[0:64, :])
                        yus[h] = yu
                return yus

            def emit_norm_apply(g, pr, yus):
                for i in range(2):
                    h = 2 * pr + i
                    qt, qp = h // 2, (h % 2) * 64
                    psb = psG.tile([64, GW], f32, tag="g", name="psb")
                    nc.tensor.matmul(
                        psb[:, :], ones65[32 * i:32 * i + 1, :],
                        rec1s[(g, h)][32 * i:32 * i + 1, :], start=True, stop=True)
                    nc.vector.tensor_mul(
                        yT[qt][qp:qp + 64, g * GW:(g + 1) * GW],
                        yus[h][:, :], psb[:, :])

            # ---- group-0 prelude: first qk GEMMs, bias tiles, v tiles ----
            yus_all = {0: {}, 1: {}}
            emit_qk(0, 0)
            emit_qk(6, 0)

            # bias f16 casts + broadcast tiles (bqv32/bp32 landed long ago)
            bqv16 = consts.tile([1, C], f16, tag="bqv16")
            nc.vector.tensor_copy(bqv16[:, :], bqv32[:, :])
            bp16 = consts.tile([1, C], f16, tag="bp16")
            nc.vector.tensor_copy(bp16[:, :], bp32[:, :])
            vb_ps = psG.tile([128, 512], f32, tag="g", name="vb_ps")
            pbias = consts.tile([128, C], f32, tag="pbias")
            vbias = consts.tile([128, C], f16, tag="vbias")
            for c0 in (0, 512):
                w = min(512, C - c0)
                nc.tensor.matmul(
                    vb_ps[:, 0:w], ones16[0:1, 0:128],
                    bqv16[0:1, c0:c0 + w], start=True, stop=True)
                nc.vector.tensor_copy(vbias[:, c0:c0 + w], vb_ps[:, 0:w])
            for c0 in (0, 512):
                w = min(512, C - c0)
                pb_ps = psG.tile([128, 512], f32, tag="g", name="pb_ps")
                nc.tensor.matmul(
                    pb_ps[:, 0:w], ones16[0:1, 0:128],
                    bp16[0:1, c0:c0 + w], start=True, stop=True)
                nc.vector.tensor_copy(pbias[:, c0:c0 + w], pb_ps[:, 0:w])

            for mi, mv in enumerate(range(12, 18)):
                cast_w(mv, mi % 2 == 0)
            for t in range(4, MT):
                emit_x16(t)
            for t in range(4):
                emit_v(t)
            cast_w(1, True)
            cast_w(7, True)
            emit_qk(1, 0)
            emit_qk(7, 0)

            # ---- group-0 pair loop ----
            fill_cast = {0: (2, 8), 1: (3, 9), 2: (4, 10), 3: (5, 11)}
            fill_qk0 = {1: (2, 8), 2: (3, 9), 3: (4, 10), 4: (5, 11)}
            fill_v = {2: 4, 3: 5, 4: 6, 5: 7}
            fill_wp = {2: (0, 1, 2), 3: (3, 4, 5)}
            g0_fillers = {
                0: (lambda: emit_transpose(4)),
                1: (lambda: emit_transpose(5)),
                2: (lambda: emit_transpose(6)),
                3: (lambda: emit_qk(1, 1)),
                4: (lambda: emit_qk(2, 1)),
                5: (lambda: emit_qk(3, 1)),
            }
            fill_qk1post = {2: (0, 6), 3: (7,), 4: (8,), 5: (9,)}
            for pr in range(6):
                yus_all[0].update(emit_pair(0, pr, filler=g0_fillers[pr]))
                if pr == 2:
                    emit_transpose(7)
                for mi, m in enumerate(fill_cast.get(pr, ())):
                    cast_w(m, mi % 2 == 0)
                for m in fill_qk0.get(pr, ()):
                    emit_qk(m, 0, bias_dve=True)
                for m in fill_qk1post.get(pr, ()):
                    emit_qk(m, 1)
                if pr in fill_v:
                    emit_v(fill_v[pr])
                for k in fill_wp.get(pr, ()):
                    load_wp(k)
                # delay each norm apply by one pair: its psb matmul waits on
                # the DVE reciprocal, so give it PE-filler lead time.
                if pr >= 1:
                    emit_norm_apply(0, pr - 1, yus_all[0])
            for m in (4, 10, 5, 11):
                emit_qk(m, 1)
            emit_norm_apply(0, 5, yus_all[0])

            # ---- group-1 pair loop ----
            for pr in range(6):
                if pr < 4:
                    filler = lambda t=pr: emit_proj_ks(t, [0, 1, 2])
                elif pr == 4:
                    filler = lambda: emit_proj_partial(5, [0, 1, 2, 3])
                else:
                    filler = lambda: emit_proj_partial(6, [0, 1, 2, 3, 4])
                yus_all[1].update(emit_pair(1, pr, filler=filler))
                if pr < 4:
                    emit_proj_ks(pr, [3, 4, 5])
                    emit_proj_fin(pr)
                    emit_norm_apply(1, pr, yus_all[1])
                elif pr == 4:
                    emit_proj_partial(4, [0, 1, 2, 3])
                    emit_norm_apply(1, 4, yus_all[1])
            emit_proj_partial(7, [0, 1, 2, 3, 4])
            emit_norm_apply(1, 5, yus_all[1])
            emit_proj_final(4, [4, 5])
            emit_proj_final(5, [4, 5])
            emit_proj_final(6, [5])
            emit_proj_final(7, [5])

    nc.finalize()
    return nc


_CACHE = {}


def _get_nc():
    if "nc" not in _CACHE:
        _CACHE["nc"] = build_nc()
    return _CACHE["nc"]


def run(inputs, trace=False):
    nc = _get_nc()
    x = np.asarray(inputs["x"], dtype=np.float32)
    in_maps = [
        {
            "x": np.ascontiguousarray(x[i]),
            "W_attn": np.asarray(inputs["W_attn"], dtype=np.float32),
            "b_attn": np.asarray(inputs["b_attn"], dtype=np.float32),
            "W_proj": np.asarray(inputs["W_proj"], dtype=np.float32),
            "b_proj": np.asarray(inputs["b_proj"], dtype=np.float32),
        }
        for i in range(B)
    ]
    res = run_bass_kernel_spmd(nc, in_maps, core_ids=list(range(B)), trace=trace)
    y = np.stack([res.results[i]["out"] for i in range(B)], axis=0)
    return y, res


def kernel(**inputs):
    y, _ = run(inputs, trace=False)
    return y


# revision 55
# speedup vs baseline: 1.0068x; 1.0038x over previous
"""Causal self-attention (transformer block) on 8 trn2 NeuronCores.

Data-parallel over batch: core i processes batch element i (B=8).
Per-core dataflow (T=1024, C=768, H=12 heads, hd=64), all matmul
operands fp16 with fp32 PSUM accumulation:

  x [T,C] --PE transpose--> xT [C,T]            (feature-major)
  qkT[m]  = W_attn[:,m].T @ xT + b  [1536,T]    (feature-major q,k)
  v[t]    = xT[t].T @ W_attn[:,v] + b [T,768]   (row-major, +ones col)
  S^T[j,i] = sum_d kT[d,j] qT[d,i]              (scores transposed)
  E = exp(S^T*scale) * causal_mask              (masked -> exact 0)
  psY = sum_j [v_j | 1].T @ E_j                 (row 64 = softmax denom)
  yT = psY[0:64] * bcast(1/psY[64])             (feature-major y)
  out[t] = yT[:,t].T @ W_proj + b               (row-major, DMA out)

Schedule highlights (engine queues are FIFO; emission order is the
schedule):
- causal suffix trim: for diagonal key tiles only query columns
  >= 128*cd are touched (S matmul, exp, AV) -- the fully-masked prefix
  is never computed and E never needs zero-filling.
- AV is split into an unmasked-suffix matmul (fires right after exp)
  plus the 128-wide diagonal block (after the DVE mask multiply), so
  the mask is off the PE critical path.
- per-(head-pair) softmax normalization; the psb broadcast (1/den) and
  yT multiplies are emitted one pair late in group 0 so the psb matmul
  always has a pair of PE work between it and the DVE reciprocal.
- projection of token tiles 0..3 runs inside the group-1 pair loop;
  tiles 4..7 partial-accumulate k<=3/4 into f16 SBUF during pairs 4/5
  so only k=4..5 chunks and evacuations trail the final norm (keeps
  the PE HAM duty-cycle warm through the tail).
- W_attn is DMA'd in [768,128] f32 column slices (order 0,6,v,1,7,...)
  so the first qk GEMMs start as early as possible; fp16 casts of the
  slices are spread over ACT/DVE as fillers.
"""
import numpy as np

import concourse.bass as bass
import concourse.tile as tile
from concourse import bacc, mybir
from concourse.bass_utils import run_bass_kernel_spmd
from concourse.masks import make_identity

f32 = mybir.dt.float32
f32r = mybir.dt.float32r
f16 = mybir.dt.float16
Exp = mybir.ActivationFunctionType.Exp

B = 8
T = 1024
C = 768
H = 12
HD = 64
SCALE = HD ** -0.5
KC = C // 128        # 6 feature chunks
MT = T // 128        # 8 token tiles
GW = 512             # Tq group width
NG = T // GW         # 2 groups
VS = HD + 1          # per-head stride in v tile (ones column at position 64)
VW = H * VS          # v tile width incl. ones column (780)


def build_nc():
    nc = bacc.Bacc(None)
    x = nc.dram_tensor("x", [T, C], f32, kind="ExternalInput")
    W_attn = nc.dram_tensor("W_attn", [C, 3 * C], f32, kind="ExternalInput")
    b_attn = nc.dram_tensor("b_attn", [3 * C], f32, kind="ExternalInput")
    W_proj = nc.dram_tensor("W_proj", [C, C], f32, kind="ExternalInput")
    b_proj = nc.dram_tensor("b_proj", [C], f32, kind="ExternalInput")
    out = nc.dram_tensor("out", [T, C], f32, kind="ExternalOutput")

    with tile.TileContext(nc) as tc:
        with (
            tc.tile_pool(name="consts", bufs=1) as consts,
            tc.tile_pool(name="stage", bufs=4) as stage,
            tc.tile_pool(name="wstage", bufs=6) as wstage,
            tc.tile_pool(name="x16p", bufs=7) as x16p,
            tc.tile_pool(name="wq", bufs=1) as wq,
            tc.tile_pool(name="wp", bufs=1) as wp,
            tc.tile_pool(name="big", bufs=1) as big,
            tc.tile_pool(name="ep", bufs=3) as ep,
            tc.tile_pool(name="small", bufs=3) as small,
            tc.tile_pool(name="outp", bufs=2) as outp,
            tc.tile_pool(name="oacc", bufs=1) as oacc,
            tc.tile_pool(name="yup", bufs=4) as yup,
            tc.tile_pool(name="psG", bufs=2, space="PSUM") as psG,
            tc.tile_pool(name="psA", bufs=2, space="PSUM") as psA,
            tc.tile_pool(name="psY", bufs=2, space="PSUM") as psY,
        ):
            # ---- first x tile on the (idle) ACT hardware-DGE queue so its
            # descriptor issue overlaps the sync queue's own first issues
            x32s = {}
            x32_t0 = stage.tile([128, C], f32, tag="x32", name="x32_t0")
            nc.scalar.dma_start(out=x32_t0[:, :], in_=x[0:128, :])
            x32s[0] = x32_t0

            # ---- ACT table warm-up: the ~1.5us activation-table load runs
            # during the startup DMA waits, not before the first real ACT op
            warm16 = consts.tile([1, 16], f16, tag="warm16")
            nc.vector.memset(warm16[:, :], 1.0)
            nc.scalar.copy(warm16[0:1, 0:1], warm16[0:1, 8:9])

            # ---- gpsimd constants, then its queue is pure DMA issue ----
            ident = consts.tile([128, 128], f16, tag="ident")
            make_identity(nc, ident[:, :])

            # multiplicative causal mask for the E diagonal 128x128 block:
            # 1 where p<=f (valid), 0 elsewhere; applied to E after exp.
            mask01 = consts.tile([128, 128], f16, tag="mask01")
            nc.gpsimd.memset(mask01[:, :], 1.0)
            nc.gpsimd.affine_select(
                out=mask01[:, :], in_=mask01[:, :],
                compare_op=mybir.AluOpType.is_ge, fill=0.0,
                base=0, pattern=[[1, 128]], channel_multiplier=-1,
            )

            # ---- x f32 tiles (t=1..3) on the sync HWDGE queue ----
            def load_x(t):
                x32 = stage.tile([128, C], f32, tag="x32")
                nc.sync.dma_start(out=x32[:, :], in_=x[t * 128:(t + 1) * 128, :])
                x32s[t] = x32

            for t in range(1, 4):
                load_x(t)

            # ---- weights: f32 column slices on sync, cast on ACT/DVE ----
            Wqk = wq.tile([128, KC, 2 * C], f16, tag="Wqk", name="Wqk")
            Wv = wq.tile([128, KC, C], f16, tag="Wv", name="Wv")
            Wp16 = wp.tile([128, KC, C], f16, tag="Wp16", name="Wp16")

            x16s = {}
            w32s = {}

            def wdma(m):
                w32 = wstage.tile([128, KC, 128], f32, tag="w32")
                nc.sync.dma_start(
                    out=w32[:, :, :],
                    in_=W_attn[:, m * 128:(m + 1) * 128]
                    .rearrange("(k p) m -> p k m", p=128))
                w32s[m] = w32

            def cast_w(m, on_act):
                dst = (Wqk[:, :, m * 128:(m + 1) * 128] if m < 12
                       else Wv[:, :, (m - 12) * 128:(m - 11) * 128])
                if on_act:
                    nc.scalar.copy(dst, w32s[m][:, :, :])
                else:
                    nc.vector.tensor_copy(dst, w32s[m][:, :, :])

            def load_wp(k):
                wp32 = wstage.tile([128, C], f32, tag="wp32")
                nc.sync.dma_start(out=wp32[:, :], in_=W_proj[k * 128:(k + 1) * 128, :])
                nc.vector.tensor_copy(Wp16[:, k, :], wp32[:, :])

            wdma(0)
            wdma(6)

            # bias consts (small DMAs; <=3KB rows so descriptor gen is cheap)
            ones16 = consts.tile([1, 512], f16, tag="ones16")
            nc.vector.memset(ones16[:, :], 1.0)
            ones65 = consts.tile([65, 64], f16, tag="ones65")
            nc.vector.memset(ones65[:, :], 1.0)

            bcol = consts.tile([128, 3 * C // 128], f32, tag="bcol")
            nc.sync.dma_start(
                out=bcol[:, :],
                in_=b_attn.ap().rearrange("(m p) -> p m", p=128))
            bqv32 = consts.tile([1, C], f32, tag="bqv32")
            nc.sync.dma_start(
                out=bqv32[:, :],
                in_=b_attn[2 * C:3 * C].rearrange("(a d) -> a d", a=1))
            bp32 = consts.tile([1, C], f32, tag="bp32")
            nc.sync.dma_start(out=bp32[:, :], in_=b_proj.ap().rearrange("(a d) -> a d", a=1))

            for m in (12, 13, 14, 15, 16, 17, 1, 7):
                wdma(m)
            for t in range(4, MT):
                load_x(t)
            for m in (2, 8, 3, 9, 4, 10, 5, 11):
                wdma(m)

            # ---- x fp16 convert + transpose to xT [C, T] ----
            xT = [big.tile([128, T], f16, tag=f"xT_{c}", name=f"xT_{c}") for c in range(KC)]

            def emit_x16(t):
                x16 = x16p.tile([128, C], f16, tag="x16")
                # alternate ACT/DVE so neither engine serializes the startup
                if t % 2 == 0:
                    nc.scalar.copy(x16[:, :], x32s[t][:, :])
                else:
                    nc.vector.tensor_copy(x16[:, :], x32s[t][:, :])
                x16s[t] = x16

            tcount = [0]

            def emit_transpose(t):
                if t not in x16s:
                    emit_x16(t)
                x16 = x16s.pop(t)
                for c in range(KC):
                    i = tcount[0]
                    tcount[0] += 1
                    if t < 4:
                        # startup batch: alternate psum pool (psA is idle)
                        # and evacuation engine so PE never waits an evac
                        pool, tag = (psA, "s") if (i % 2) else (psG, "g")
                        pst = pool.tile([128, 128], f16, tag=tag, name="pst")
                        nc.tensor.transpose(
                            pst[:, :], x16[:, c * 128:(c + 1) * 128], ident[:, :])
                        if i % 2:
                            nc.scalar.copy(
                                xT[c][:, t * 128:(t + 1) * 128], pst[:, :])
                        else:
                            nc.vector.tensor_copy(
                                xT[c][:, t * 128:(t + 1) * 128], pst[:, :])
                    else:
                        # in-pair filler: psG only, DVE evac (ACT runs exp)
                        pst = psG.tile([128, 128], f16, tag="g", name="pst")
                        nc.tensor.transpose(
                            pst[:, :], x16[:, c * 128:(c + 1) * 128], ident[:, :])
                        nc.vector.tensor_copy(
                            xT[c][:, t * 128:(t + 1) * 128], pst[:, :])

            for t in range(4):
                emit_transpose(t)
            cast_w(0, True)
            cast_w(6, True)

            # ---- qk^T GEMM: qkT[m] [128, T] f16, m=0..11 covers features 0..1535
            qkT = [big.tile([128, T], f16, tag=f"qkT_{m}", name=f"qkT_{m}") for m in range(12)]

            def emit_qk(m, n, bias_dve=False):
                ps = psG.tile([128, 512], f32, tag="g", name="qk_ps")
                for k in range(KC):
                    nc.tensor.matmul(
                        ps[:, :], Wqk[:, k, m * 128:(m + 1) * 128],
                        xT[k][:, n * 512:(n + 1) * 512],
                        start=(k == 0), stop=(k == KC - 1))
                # bias is per-partition in the feature-major layout: fold it
                # into the psum->sbuf copy
                if n == 0 and not bias_dve:
                    nc.scalar.activation(
                        qkT[m][:, n * 512:(n + 1) * 512], ps[:, :],
                        mybir.ActivationFunctionType.Identity,
                        bias=bcol[:, m:m + 1])
                else:
                    nc.vector.tensor_scalar_add(
                        qkT[m][:, n * 512:(n + 1) * 512], ps[:, :],
                        bcol[:, m:m + 1])

            # ---- v rows: v_sb[t] [128, 780] f16 (64 cols + ones col per head)
            v_sb = [big.tile([128, VW], f16, tag=f"v_{t}", name=f"v_{t}") for t in range(MT)]

            def emit_v(t):
                vht = v_sb[t][:, :].rearrange("p (h s) -> p h s", s=VS)
                nc.vector.memset(vht[:, :, HD:HD + 1], 1.0)
                vchunks = ((0, 512), (512, 256))
                pss = [psG.tile([128, 512], f32, tag="g", name=f"v_ps{n}")
                       for n in range(2)]
                for k in range(KC):
                    for n, (c0, w) in enumerate(vchunks):
                        nc.tensor.matmul(
                            pss[n][:, 0:w], xT[k][:, t * 128:(t + 1) * 128],
                            Wv[:, k, c0:c0 + w],
                            start=(k == 0), stop=(k == KC - 1))
                for n, (c0, w) in enumerate(vchunks):
                    nh = w // HD
                    h0 = c0 // HD
                    nc.vector.tensor_add(
                        vht[:, h0:h0 + nh, 0:HD],
                        pss[n][:, 0:w].rearrange("p (h s) -> p h s", s=HD),
                        vbias[:, c0:c0 + w].rearrange("p (h s) -> p h s", s=HD))

            # ---- attention: S^T -> exp (-> mask) -> AV (+denom) -> normalize
            yT = [big.tile([128, T], f16, tag=f"yT_{m}", name=f"yT_{m}") for m in range(KC)]

            OCHUNKS = ((0, 512), (512, 256))
            proj_pss = {}

            def emit_proj_ks(t, ks):
                if t not in proj_pss:
                    proj_pss[t] = [psG.tile([128, 512], f32, tag="g",
                                            name=f"o_ps{n}") for n in range(2)]
                pss = proj_pss[t]
                for k in ks:
                    for n, (c0, w) in enumerate(OCHUNKS):
                        nc.tensor.matmul(
                            pss[n][:, 0:w], yT[k][:, t * 128:(t + 1) * 128],
                            Wp16[:, k, c0:c0 + w],
                            start=(k == 0), stop=(k == KC - 1))

            def emit_proj_fin(t):
                pss = proj_pss.pop(t)
                o_sb = outp.tile([128, C], f32, tag="o", name="o_sb")
                for n, (c0, w) in enumerate(OCHUNKS):
                    nc.vector.tensor_add(
                        o_sb[:, c0:c0 + w], pss[n][:, 0:w], pbias[:, c0:c0 + w])
                    nc.sync.dma_start(
                        out=out[t * 128:(t + 1) * 128, c0:c0 + w],
                        in_=o_sb[:, c0:c0 + w])

            # partial projection for the tail tiles: accumulate k chunks ks
            # into PSUM, then fold (+ pbias) into the f16 SBUF accumulator.
            o_acc = {t: oacc.tile([128, C], f16, tag=f"oacc_{t}", name=f"oacc_{t}")
                     for t in range(4, MT)}

            def emit_partial_ks(t, ks, k0, k1):
                # accumulate chunks ks of the k0..k1 partial group for tile t
                key = ("p", t)
                if key not in proj_pss:
                    proj_pss[key] = [psG.tile([128, 512], f32, tag="g",
                                              name=f"op_ps{n}") for n in range(2)]
                pss = proj_pss[key]
                for k in ks:
                    for n, (c0, w) in enumerate(OCHUNKS):
                        nc.tensor.matmul(
                            pss[n][:, 0:w], yT[k][:, t * 128:(t + 1) * 128],
                            Wp16[:, k, c0:c0 + w],
                            start=(k == k0), stop=(k == k1))

            def emit_partial_evac(t):
                pss = proj_pss.pop(("p", t))
                for n, (c0, w) in enumerate(OCHUNKS):
                    nc.vector.tensor_add(
                        o_acc[t][:, c0:c0 + w], pss[n][:, 0:w], pbias[:, c0:c0 + w])

            def emit_proj_partial(t, ks):
                emit_partial_ks(t, ks, ks[0], ks[-1])
                emit_partial_evac(t)

            def emit_proj_final(t, ks):
                pss = [psG.tile([128, 512], f32, tag="g", name=f"of_ps{n}")
                       for n in range(2)]
                for k in ks:
                    for n, (c0, w) in enumerate(OCHUNKS):
                        nc.tensor.matmul(
                            pss[n][:, 0:w], yT[k][:, t * 128:(t + 1) * 128],
                            Wp16[:, k, c0:c0 + w],
                            start=(k == ks[0]), stop=(k == ks[-1]))
                o_sb = outp.tile([128, C], f32, tag="o", name="o_sb")
                for n, (c0, w) in enumerate(OCHUNKS):
                    nc.vector.tensor_add(
                        o_sb[:, c0:c0 + w], pss[n][:, 0:w], o_acc[t][:, c0:c0 + w])
                    nc.sync.dma_start(
                        out=out[t * 128:(t + 1) * 128, c0:c0 + w],
                        in_=o_sb[:, c0:c0 + w])

            rec1s = {}

            def emit_pair(g, pr, filler=None, jfillers=None):
                hA, hB = 2 * pr, 2 * pr + 1
                qt = pr
                nchunks = 4 * g + 4
                psyA = psY.tile([65, GW], f32, tag="y", name="psyA")
                psyB = psY.tile([65, GW], f32, tag="y", name="psyB")
                for j in range(nchunks):
                    cd = j - 4 * g  # diagonal col-block index
                    c0 = max(cd, 0) * 128  # masked-prefix width: skip it
                    psS = psA.tile([128, 2, GW], f32, tag="s", name="psS")
                    nc.tensor.matmul(
                        psS[:, 0, c0:GW],
                        qkT[6 + qt][0:64, j * 128:(j + 1) * 128],
                        qkT[qt][0:64, g * GW + c0:(g + 1) * GW],
                        start=True, stop=True)
                    nc.tensor.matmul(
                        psS[:, 1, c0:GW],
                        qkT[6 + qt][64:128, j * 128:(j + 1) * 128],
                        qkT[qt][64:128, g * GW + c0:(g + 1) * GW],
                        start=True, stop=True)
                    E2 = ep.tile([128, 2, GW], f16, tag="e", name="E2")
                    nc.scalar.activation(
                        E2[:, :, c0:GW], psS[:, :, c0:GW], Exp, scale=SCALE)
                    if cd >= 0:
                        # zero the strictly-upper triangle of the diagonal
                        # block (DVE); the unmasked AV suffix below does not
                        # wait for it.
                        nc.vector.tensor_mul(
                            E2[:, 0:2, c0:c0 + 128], E2[:, 0:2, c0:c0 + 128],
                            mask01[:, :].unsqueeze(1).broadcast_to((128, 2, 128)))
                    if j == 0 and filler is not None:
                        # PE filler between the first S pair and its AV (the
                        # AV waits on the exp latency at pair startup).
                        filler()
                    if jfillers is not None and j in jfillers:
                        jfillers[j]()
                    last = j == nchunks - 1
                    cm = c0 + 128  # end of the diagonal (masked) block
                    for ei, (h, psy) in enumerate(((hA, psyA), (hB, psyB))):
                        vsl = v_sb[j][:, h * VS:h * VS + HD + 1]
                        if cd >= 0:
                            if cm < GW:
                                # unmasked suffix: no mask dependency
                                nc.tensor.matmul(
                                    psy[:, cm:GW], vsl, E2[:, ei, cm:GW],
                                    start=(j == 0), stop=False,
                                    skip_group_check=True)
                            # masked diagonal block (waits on the DVE mask)
                            nc.tensor.matmul(
                                psy[:, c0:cm], vsl, E2[:, ei, c0:cm],
                                start=False, stop=last,
                                skip_group_check=True)
                        else:
                            nc.tensor.matmul(
                                psy[:, :], vsl, E2[:, ei, :],
                                start=(j == 0), stop=last,
                                skip_group_check=True)
                # readout: yu + den per head (psY recycles as early as
                # possible), then the shared reciprocal chain. For the LAST
                # pair there is no next pair waiting on psY, so the
                # denominators go first and the tail psb matmul waits ~1.3us
                # less.
                yus = {}
                den2 = small.tile([33, GW], f32, tag="den2", name=f"den2_{g}_{pr}")
                den_first = (g, pr) == (1, 5)
                if den_first:
                    # whole reciprocal chain ahead of the y copies in the
                    # DVE FIFO: the tail psb matmul waits ~1.3us less
                    nc.vector.tensor_copy(den2[0:1, :], psyA[64:65, :])
                    nc.vector.tensor_copy(den2[32:33, :], psyB[64:65, :])
                else:
                    for i, (h, psy) in enumerate(((hA, psyA), (hB, psyB))):
                        yu = yup.tile([64, GW], f16, tag="yu", name="yu")
                        nc.vector.tensor_copy(yu[:, :], psy[0:64, :])
                        nc.vector.tensor_copy(den2[32 * i:32 * i + 1, :], psy[64:65, :])
                        yus[h] = yu
                rec2 = small.tile([33, GW], f32, tag="rec2", name=f"rec2_{g}_{pr}")
                nc.vector.reciprocal_approx_fast(out=rec2[:, :], in_=den2[:, :])
                rec16 = small.tile([33, GW], f16, tag="rec16", name=f"rec16_{g}_{pr}")
                nc.vector.tensor_copy(rec16[:, :], rec2[:, :])
                rec1s[(g, 2 * pr)] = rec16
                rec1s[(g, 2 * pr + 1)] = rec16
                if den_first:
                    for i, (h, psy) in enumerate(((hA, psyA), (hB, psyB))):
                        yu = yup.tile([64, GW], f16, tag="yu", name="yu")
                        nc.vector.tensor_copy(yu[:, :], psy[0:64, :])
                        yus[h] = yu
                return yus

            def emit_norm_apply(g, pr, yus):
                for i in range(2):
                    h = 2 * pr + i
                    qt, qp = h // 2, (h % 2) * 64
                    psb = psG.tile([64, GW], f32, tag="g", name="psb")
                    nc.tensor.matmul(
                        psb[:, :], ones65[32 * i:32 * i + 1, :],
                        rec1s[(g, h)][32 * i:32 * i + 1, :], start=True, stop=True)
                    nc.vector.tensor_mul(
                        yT[qt][qp:qp + 64, g * GW:(g + 1) * GW],
                        yus[h][:, :], psb[:, :])

            # ---- group-0 prelude: first qk GEMMs, bias tiles, v tiles ----
            yus_all = {0: {}, 1: {}}
            emit_qk(0, 0)
            emit_qk(6, 0)

            # bias f16 casts + broadcast tiles (bqv32/bp32 landed long ago)
            bqv16 = consts.tile([1, C], f16, tag="bqv16")
            nc.vector.tensor_copy(bqv16[:, :], bqv32[:, :])
            bp16 = consts.tile([1, C], f16, tag="bp16")
            nc.vector.tensor_copy(bp16[:, :], bp32[:, :])
            vb_ps = psG.tile([128, 512], f32, tag="g", name="vb_ps")
            pbias = consts.tile([128, C], f32, tag="pbias")
            vbias = consts.tile([128, C], f16, tag="vbias")
            for c0 in (0, 512):
                w = min(512, C - c0)
                nc.tensor.matmul(
                    vb_ps[:, 0:w], ones16[0:1, 0:128],
                    bqv16[0:1, c0:c0 + w], start=True, stop=True)
                nc.vector.tensor_copy(vbias[:, c0:c0 + w], vb_ps[:, 0:w])
            for c0 in (0, 512):
                w = min(512, C - c0)
                pb_ps = psG.tile([128, 512], f32, tag="g", name="pb_ps")
                nc.tensor.matmul(
                    pb_ps[:, 0:w], ones16[0:1, 0:128],
                    bp16[0:1, c0:c0 + w], start=True, stop=True)
                nc.vector.tensor_copy(pbias[:, c0:c0 + w], pb_ps[:, 0:w])

            for mi, mv in enumerate(range(12, 18)):
                cast_w(mv, mi % 2 == 0)
            for t in range(4, MT):
                emit_x16(t)
            for t in range(4):
                emit_v(t)
            cast_w(1, True)
            cast_w(7, True)
            emit_qk(1, 0)
            emit_qk(7, 0)

            # ---- group-0 pair loop ----
            fill_cast = {0: (2, 8), 1: (3, 9), 2: (4, 10), 3: (5, 11)}
            fill_qk0 = {1: (2, 8), 2: (3, 9), 3: (4, 10), 4: (5, 11)}
            fill_v = {2: 4, 3: 5, 4: 6, 5: 7}
            fill_wp = {2: (0, 1, 2), 3: (3, 4, 5)}
            g0_fillers = {
                0: (lambda: emit_transpose(4)),
                1: (lambda: emit_transpose(5)),
                2: (lambda: emit_transpose(6)),
                3: (lambda: emit_qk(1, 1)),
                4: (lambda: emit_qk(2, 1)),
                5: (lambda: emit_qk(3, 1)),
            }
            fill_qk1post = {2: (0, 6), 3: (7,), 4: (8,), 5: (9,)}
            for pr in range(6):
                yus_all[0].update(emit_pair(0, pr, filler=g0_fillers[pr]))
                if pr == 2:
                    emit_transpose(7)
                for mi, m in enumerate(fill_cast.get(pr, ())):
                    cast_w(m, mi % 2 == 0)
                for m in fill_qk0.get(pr, ()):
                    emit_qk(m, 0, bias_dve=True)
                for m in fill_qk1post.get(pr, ()):
                    emit_qk(m, 1)
                if pr in fill_v:
                    emit_v(fill_v[pr])
                for k in fill_wp.get(pr, ()):
                    load_wp(k)
                # delay each norm apply by one pair: its psb matmul waits on
                # the DVE reciprocal, so give it PE-filler lead time.
                if pr >= 1:
                    emit_norm_apply(0, pr - 1, yus_all[0])
            for m in (4, 10, 5, 11):
                emit_qk(m, 1)
            emit_norm_apply(0, 5, yus_all[0])

            # ---- group-1 pair loop ----
            for pr in range(6):
                if pr < 4:
                    filler = lambda t=pr: emit_proj_ks(t, [0, 1, 2])
                elif pr == 4:
                    filler = lambda: emit_proj_partial(5, [0, 1, 2, 3])
                else:
                    filler = lambda: emit_proj_partial(6, [0, 1, 2, 3, 4])
                yus_all[1].update(emit_pair(1, pr, filler=filler))
                if pr < 4:
                    emit_proj_ks(pr, [3, 4, 5])
                    emit_proj_fin(pr)
                    emit_norm_apply(1, pr, yus_all[1])
                elif pr == 4:
                    emit_proj_partial(4, [0, 1, 2, 3])
                    emit_norm_apply(1, 4, yus_all[1])
            emit_proj_partial(7, [0, 1, 2, 3, 4])
            emit_norm_apply(1, 5, yus_all[1])
            emit_proj_final(4, [4, 5])
            emit_proj_final(5, [4, 5])
            emit_proj_final(6, [5])
            emit_proj_final(7, [5])

    nc.finalize()
    return nc


_CACHE = {}


def _get_nc():
    if "nc" not in _CACHE:
        _CACHE["nc"] = build_nc()
    return _CACHE["nc"]


def run(inputs, trace=False):
    nc = _get_nc()
    x = np.asarray(inputs["x"], dtype=np.float32)
    in_maps = [
        {
            "x": np.ascontiguousarray(x[i]),
            "W_attn": np.asarray(inputs["W_attn"], dtype=np.float32),
            "b_attn": np.asarray(inputs["b_attn"], dtype=np.float32),
            "W_proj": np.asarray(inputs["W_proj"], dtype=np.float32),
            "b_proj": np.asarray(inputs["b_proj"], dtype=np.float32),
        }
        for i in range(B)
    ]
    res = run_bass_kernel_spmd(nc, in_maps, core_ids=list(range(B)), trace=trace)
    y = np.stack([res.results[i]["out"] for i in range(B)], axis=0)
    return y, res


def kernel(**inputs):
    y, _ = run(inputs, trace=False)
    return y


# revision 56
# speedup vs baseline: 1.0100x; 1.0031x over previous
"""Causal self-attention (transformer block) on 8 trn2 NeuronCores.

Data-parallel over batch: core i processes batch element i (B=8).
Per-core dataflow (T=1024, C=768, H=12 heads, hd=64), all matmul
operands fp16 with fp32 PSUM accumulation:

  x [T,C] --PE transpose--> xT [C,T]            (feature-major)
  qkT[m]  = W_attn[:,m].T @ xT + b  [1536,T]    (feature-major q,k)
  v[t]    = xT[t].T @ W_attn[:,v] + b [T,768]   (row-major, +ones col)
  S^T[j,i] = sum_d kT[d,j] qT[d,i]              (scores transposed)
  E = exp(S^T*scale) * causal_mask              (masked -> exact 0)
  psY = sum_j [v_j | 1].T @ E_j                 (row 64 = softmax denom)
  yT = psY[0:64] * bcast(1/psY[64])             (feature-major y)
  out[t] = yT[:,t].T @ W_proj + b               (row-major, DMA out)

Schedule highlights (engine queues are FIFO; emission order is the
schedule):
- causal suffix trim: for diagonal key tiles only query columns
  >= 128*cd are touched (S matmul, exp, AV) -- the fully-masked prefix
  is never computed and E never needs zero-filling.
- AV is split into an unmasked-suffix matmul (fires right after exp)
  plus the 128-wide diagonal block (after the DVE mask multiply), so
  the mask is off the PE critical path.
- per-(head-pair) softmax normalization; the psb broadcast (1/den) and
  yT multiplies are emitted one pair late in group 0 so the psb matmul
  always has a pair of PE work between it and the DVE reciprocal.
- projection of token tiles 0..3 runs inside the group-1 pair loop;
  tiles 4..7 partial-accumulate k<=3/4 into f16 SBUF during pairs 4/5
  so only k=4..5 chunks and evacuations trail the final norm (keeps
  the PE HAM duty-cycle warm through the tail).
- W_attn is DMA'd in [768,128] f32 column slices (order 0,6,v,1,7,...)
  so the first qk GEMMs start as early as possible; fp16 casts of the
  slices are spread over ACT/DVE as fillers.
"""
import numpy as np

import concourse.bass as bass
import concourse.tile as tile
from concourse import bacc, mybir
from concourse.bass_utils import run_bass_kernel_spmd
from concourse.masks import make_identity

f32 = mybir.dt.float32
f32r = mybir.dt.float32r
f16 = mybir.dt.float16
Exp = mybir.ActivationFunctionType.Exp

B = 8
T = 1024
C = 768
H = 12
HD = 64
SCALE = HD ** -0.5
KC = C // 128        # 6 feature chunks
MT = T // 128        # 8 token tiles
GW = 512             # Tq group width
NG = T // GW         # 2 groups
VS = HD + 1          # per-head stride in v tile (ones column at position 64)
VW = H * VS          # v tile width incl. ones column (780)


def build_nc():
    nc = bacc.Bacc(None)
    x = nc.dram_tensor("x", [T, C], f32, kind="ExternalInput")
    W_attn = nc.dram_tensor("W_attn", [C, 3 * C], f32, kind="ExternalInput")
    b_attn = nc.dram_tensor("b_attn", [3 * C], f32, kind="ExternalInput")
    W_proj = nc.dram_tensor("W_proj", [C, C], f32, kind="ExternalInput")
    b_proj = nc.dram_tensor("b_proj", [C], f32, kind="ExternalInput")
    out = nc.dram_tensor("out", [T, C], f32, kind="ExternalOutput")

    with tile.TileContext(nc) as tc:
        with (
            tc.tile_pool(name="consts", bufs=1) as consts,
            tc.tile_pool(name="stage", bufs=3) as stage,
            tc.tile_pool(name="wstage", bufs=6) as wstage,
            tc.tile_pool(name="x16p", bufs=7) as x16p,
            tc.tile_pool(name="wq", bufs=1) as wq,
            tc.tile_pool(name="wp", bufs=1) as wp,
            tc.tile_pool(name="big", bufs=1) as big,
            tc.tile_pool(name="ep", bufs=4) as ep,
            tc.tile_pool(name="small", bufs=3) as small,
            tc.tile_pool(name="outp", bufs=2) as outp,
            tc.tile_pool(name="oacc", bufs=1) as oacc,
            tc.tile_pool(name="yup", bufs=4) as yup,
            tc.tile_pool(name="psG", bufs=2, space="PSUM") as psG,
            tc.tile_pool(name="psA", bufs=2, space="PSUM") as psA,
            tc.tile_pool(name="psY", bufs=2, space="PSUM") as psY,
        ):
            # ---- first x tile on the (idle) ACT hardware-DGE queue so its
            # descriptor issue overlaps the sync queue's own first issues
            x32s = {}
            x32_t0 = stage.tile([128, C], f32, tag="x32", name="x32_t0")
            nc.scalar.dma_start(out=x32_t0[:, :], in_=x[0:128, :])
            x32s[0] = x32_t0

            # ---- ACT table warm-up: the ~1.5us activation-table load runs
            # during the startup DMA waits, not before the first real ACT op
            warm16 = consts.tile([1, 16], f16, tag="warm16")
            nc.vector.memset(warm16[:, :], 1.0)
            nc.scalar.copy(warm16[0:1, 0:1], warm16[0:1, 8:9])

            # ---- gpsimd constants, then its queue is pure DMA issue ----
            ident = consts.tile([128, 128], f16, tag="ident")
            make_identity(nc, ident[:, :])

            # multiplicative causal mask for the E diagonal 128x128 block:
            # 1 where p<=f (valid), 0 elsewhere; applied to E after exp.
            mask01 = consts.tile([128, 128], f16, tag="mask01")
            nc.gpsimd.memset(mask01[:, :], 1.0)
            nc.gpsimd.affine_select(
                out=mask01[:, :], in_=mask01[:, :],
                compare_op=mybir.AluOpType.is_ge, fill=0.0,
                base=0, pattern=[[1, 128]], channel_multiplier=-1,
            )

            # ---- x f32 tiles (t=1..3) on the sync HWDGE queue ----
            def load_x(t):
                x32 = stage.tile([128, C], f32, tag="x32")
                nc.sync.dma_start(out=x32[:, :], in_=x[t * 128:(t + 1) * 128, :])
                x32s[t] = x32

            for t in range(1, 4):
                load_x(t)

            # ---- weights: f32 column slices on sync, cast on ACT/DVE ----
            Wqk = wq.tile([128, KC, 2 * C], f16, tag="Wqk", name="Wqk")
            Wv = wq.tile([128, KC, C], f16, tag="Wv", name="Wv")
            Wp16 = wp.tile([128, KC, C], f16, tag="Wp16", name="Wp16")

            x16s = {}
            w32s = {}

            def wdma(m):
                w32 = wstage.tile([128, KC, 128], f32, tag="w32")
                nc.sync.dma_start(
                    out=w32[:, :, :],
                    in_=W_attn[:, m * 128:(m + 1) * 128]
                    .rearrange("(k p) m -> p k m", p=128))
                w32s[m] = w32

            def cast_w(m, on_act):
                dst = (Wqk[:, :, m * 128:(m + 1) * 128] if m < 12
                       else Wv[:, :, (m - 12) * 128:(m - 11) * 128])
                if on_act:
                    nc.scalar.copy(dst, w32s[m][:, :, :])
                else:
                    nc.vector.tensor_copy(dst, w32s[m][:, :, :])

            def load_wp(k):
                wp32 = wstage.tile([128, C], f32, tag="wp32")
                nc.sync.dma_start(out=wp32[:, :], in_=W_proj[k * 128:(k + 1) * 128, :])
                nc.vector.tensor_copy(Wp16[:, k, :], wp32[:, :])

            wdma(0)
            wdma(6)

            # bias consts (small DMAs; <=3KB rows so descriptor gen is cheap)
            ones16 = consts.tile([1, 512], f16, tag="ones16")
            nc.vector.memset(ones16[:, :], 1.0)
            ones65 = consts.tile([65, 64], f16, tag="ones65")
            nc.vector.memset(ones65[:, :], 1.0)

            bcol = consts.tile([128, 3 * C // 128], f32, tag="bcol")
            nc.sync.dma_start(
                out=bcol[:, :],
                in_=b_attn.ap().rearrange("(m p) -> p m", p=128))
            bqv32 = consts.tile([1, C], f32, tag="bqv32")
            nc.sync.dma_start(
                out=bqv32[:, :],
                in_=b_attn[2 * C:3 * C].rearrange("(a d) -> a d", a=1))
            bp32 = consts.tile([1, C], f32, tag="bp32")
            nc.sync.dma_start(out=bp32[:, :], in_=b_proj.ap().rearrange("(a d) -> a d", a=1))

            for m in (12, 13, 14, 15, 16, 17, 1, 7):
                wdma(m)
            for t in range(4, MT):
                load_x(t)
            for m in (2, 8, 3, 9, 4, 10, 5, 11):
                wdma(m)

            # ---- x fp16 convert + transpose to xT [C, T] ----
            xT = [big.tile([128, T], f16, tag=f"xT_{c}", name=f"xT_{c}") for c in range(KC)]

            def emit_x16(t):
                x16 = x16p.tile([128, C], f16, tag="x16")
                # alternate ACT/DVE so neither engine serializes the startup
                if t % 2 == 0:
                    nc.scalar.copy(x16[:, :], x32s[t][:, :])
                else:
                    nc.vector.tensor_copy(x16[:, :], x32s[t][:, :])
                x16s[t] = x16

            tcount = [0]

            def emit_transpose(t):
                if t not in x16s:
                    emit_x16(t)
                x16 = x16s.pop(t)
                for c in range(KC):
                    i = tcount[0]
                    tcount[0] += 1
                    if t < 4:
                        # startup batch: alternate psum pool (psA is idle)
                        # and evacuation engine so PE never waits an evac
                        pool, tag = (psA, "s") if (i % 2) else (psG, "g")
                        pst = pool.tile([128, 128], f16, tag=tag, name="pst")
                        nc.tensor.transpose(
                            pst[:, :], x16[:, c * 128:(c + 1) * 128], ident[:, :])
                        if i % 2:
                            nc.scalar.copy(
                                xT[c][:, t * 128:(t + 1) * 128], pst[:, :])
                        else:
                            nc.vector.tensor_copy(
                                xT[c][:, t * 128:(t + 1) * 128], pst[:, :])
                    else:
                        # in-pair filler: psG only, DVE evac (ACT runs exp)
                        pst = psG.tile([128, 128], f16, tag="g", name="pst")
                        nc.tensor.transpose(
                            pst[:, :], x16[:, c * 128:(c + 1) * 128], ident[:, :])
                        nc.vector.tensor_copy(
                            xT[c][:, t * 128:(t + 1) * 128], pst[:, :])

            for t in range(4):
                emit_transpose(t)
            cast_w(0, True)
            cast_w(6, True)

            # ---- qk^T GEMM: qkT[m] [128, T] f16, m=0..11 covers features 0..1535
            qkT = [big.tile([128, T], f16, tag=f"qkT_{m}", name=f"qkT_{m}") for m in range(12)]

            def emit_qk(m, n, bias_dve=False):
                ps = psG.tile([128, 512], f32, tag="g", name="qk_ps")
                for k in range(KC):
                    nc.tensor.matmul(
                        ps[:, :], Wqk[:, k, m * 128:(m + 1) * 128],
                        xT[k][:, n * 512:(n + 1) * 512],
                        start=(k == 0), stop=(k == KC - 1))
                # bias is per-partition in the feature-major layout: fold it
                # into the psum->sbuf copy
                if n == 0 and not bias_dve:
                    nc.scalar.activation(
                        qkT[m][:, n * 512:(n + 1) * 512], ps[:, :],
                        mybir.ActivationFunctionType.Identity,
                        bias=bcol[:, m:m + 1])
                else:
                    nc.vector.tensor_scalar_add(
                        qkT[m][:, n * 512:(n + 1) * 512], ps[:, :],
                        bcol[:, m:m + 1])

            # ---- v rows: v_sb[t] [128, 780] f16 (64 cols + ones col per head)
            v_sb = [big.tile([128, VW], f16, tag=f"v_{t}", name=f"v_{t}") for t in range(MT)]

            def emit_v(t):
                vht = v_sb[t][:, :].rearrange("p (h s) -> p h s", s=VS)
                nc.vector.memset(vht[:, :, HD:HD + 1], 1.0)
                vchunks = ((0, 512), (512, 256))
                pss = [psG.tile([128, 512], f32, tag="g", name=f"v_ps{n}")
                       for n in range(2)]
                for k in range(KC):
                    for n, (c0, w) in enumerate(vchunks):
                        nc.tensor.matmul(
                            pss[n][:, 0:w], xT[k][:, t * 128:(t + 1) * 128],
                            Wv[:, k, c0:c0 + w],
                            start=(k == 0), stop=(k == KC - 1))
                for n, (c0, w) in enumerate(vchunks):
                    nh = w // HD
                    h0 = c0 // HD
                    nc.vector.tensor_add(
                        vht[:, h0:h0 + nh, 0:HD],
                        pss[n][:, 0:w].rearrange("p (h s) -> p h s", s=HD),
                        vbias[:, c0:c0 + w].rearrange("p (h s) -> p h s", s=HD))

            # ---- attention: S^T -> exp (-> mask) -> AV (+denom) -> normalize
            yT = [big.tile([128, T], f16, tag=f"yT_{m}", name=f"yT_{m}") for m in range(KC)]

            OCHUNKS = ((0, 512), (512, 256))
            proj_pss = {}

            def emit_proj_ks(t, ks):
                if t not in proj_pss:
                    proj_pss[t] = [psG.tile([128, 512], f32, tag="g",
                                            name=f"o_ps{n}") for n in range(2)]
                pss = proj_pss[t]
                for k in ks:
                    for n, (c0, w) in enumerate(OCHUNKS):
                        nc.tensor.matmul(
                            pss[n][:, 0:w], yT[k][:, t * 128:(t + 1) * 128],
                            Wp16[:, k, c0:c0 + w],
                            start=(k == 0), stop=(k == KC - 1))

            def emit_proj_fin(t):
                pss = proj_pss.pop(t)
                o_sb = outp.tile([128, C], f32, tag="o", name="o_sb")
                for n, (c0, w) in enumerate(OCHUNKS):
                    nc.vector.tensor_add(
                        o_sb[:, c0:c0 + w], pss[n][:, 0:w], pbias[:, c0:c0 + w])
                    nc.sync.dma_start(
                        out=out[t * 128:(t + 1) * 128, c0:c0 + w],
                        in_=o_sb[:, c0:c0 + w])

            # partial projection for the tail tiles: accumulate k chunks ks
            # into PSUM, then fold (+ pbias) into the f16 SBUF accumulator.
            o_acc = {t: oacc.tile([128, C], f16, tag=f"oacc_{t}", name=f"oacc_{t}")
                     for t in range(4, MT)}

            def emit_partial_ks(t, ks, k0, k1):
                # accumulate chunks ks of the k0..k1 partial group for tile t
                key = ("p", t)
                if key not in proj_pss:
                    proj_pss[key] = [psG.tile([128, 512], f32, tag="g",
                                              name=f"op_ps{n}") for n in range(2)]
                pss = proj_pss[key]
                for k in ks:
                    for n, (c0, w) in enumerate(OCHUNKS):
                        nc.tensor.matmul(
                            pss[n][:, 0:w], yT[k][:, t * 128:(t + 1) * 128],
                            Wp16[:, k, c0:c0 + w],
                            start=(k == k0), stop=(k == k1))

            def emit_partial_evac(t):
                pss = proj_pss.pop(("p", t))
                for n, (c0, w) in enumerate(OCHUNKS):
                    nc.vector.tensor_add(
                        o_acc[t][:, c0:c0 + w], pss[n][:, 0:w], pbias[:, c0:c0 + w])

            def emit_proj_partial(t, ks):
                emit_partial_ks(t, ks, ks[0], ks[-1])
                emit_partial_evac(t)

            def emit_proj_final(t, ks):
                pss = [psG.tile([128, 512], f32, tag="g", name=f"of_ps{n}")
                       for n in range(2)]
                for k in ks:
                    for n, (c0, w) in enumerate(OCHUNKS):
                        nc.tensor.matmul(
                            pss[n][:, 0:w], yT[k][:, t * 128:(t + 1) * 128],
                            Wp16[:, k, c0:c0 + w],
                            start=(k == ks[0]), stop=(k == ks[-1]))
                o_sb = outp.tile([128, C], f32, tag="o", name="o_sb")
                for n, (c0, w) in enumerate(OCHUNKS):
                    nc.vector.tensor_add(
                        o_sb[:, c0:c0 + w], pss[n][:, 0:w], o_acc[t][:, c0:c0 + w])
                    nc.sync.dma_start(
                        out=out[t * 128:(t + 1) * 128, c0:c0 + w],
                        in_=o_sb[:, c0:c0 + w])

            rec1s = {}

            def emit_pair(g, pr, filler=None, jfillers=None):
                hA, hB = 2 * pr, 2 * pr + 1
                qt = pr
                nchunks = 4 * g + 4
                psyA = psY.tile([65, GW], f32, tag="y", name="psyA")
                psyB = psY.tile([65, GW], f32, tag="y", name="psyB")
                for j in range(nchunks):
                    cd = j - 4 * g  # diagonal col-block index
                    c0 = max(cd, 0) * 128  # masked-prefix width: skip it
                    psS = psA.tile([128, 2, GW], f32, tag="s", name="psS")
                    nc.tensor.matmul(
                        psS[:, 0, c0:GW],
                        qkT[6 + qt][0:64, j * 128:(j + 1) * 128],
                        qkT[qt][0:64, g * GW + c0:(g + 1) * GW],
                        start=True, stop=True)
                    nc.tensor.matmul(
                        psS[:, 1, c0:GW],
                        qkT[6 + qt][64:128, j * 128:(j + 1) * 128],
                        qkT[qt][64:128, g * GW + c0:(g + 1) * GW],
                        start=True, stop=True)
                    E2 = ep.tile([128, 2, GW], f16, tag="e", name="E2")
                    nc.scalar.activation(
                        E2[:, :, c0:GW], psS[:, :, c0:GW], Exp, scale=SCALE)
                    if cd >= 0:
                        # zero the strictly-upper triangle of the diagonal
                        # block (DVE); the unmasked AV suffix below does not
                        # wait for it.
                        nc.vector.tensor_mul(
                            E2[:, 0:2, c0:c0 + 128], E2[:, 0:2, c0:c0 + 128],
                            mask01[:, :].unsqueeze(1).broadcast_to((128, 2, 128)))
                    if j == 0 and filler is not None:
                        # PE filler between the first S pair and its AV (the
                        # AV waits on the exp latency at pair startup).
                        filler()
                    if jfillers is not None and j in jfillers:
                        jfillers[j]()
                    last = j == nchunks - 1
                    cm = c0 + 128  # end of the diagonal (masked) block
                    for ei, (h, psy) in enumerate(((hA, psyA), (hB, psyB))):
                        vsl = v_sb[j][:, h * VS:h * VS + HD + 1]
                        if cd >= 0:
                            if cm < GW:
                                # unmasked suffix: no mask dependency
                                nc.tensor.matmul(
                                    psy[:, cm:GW], vsl, E2[:, ei, cm:GW],
                                    start=(j == 0), stop=False,
                                    skip_group_check=True)
                            # masked diagonal block (waits on the DVE mask)
                            nc.tensor.matmul(
                                psy[:, c0:cm], vsl, E2[:, ei, c0:cm],
                                start=False, stop=last,
                                skip_group_check=True)
                        else:
                            nc.tensor.matmul(
                                psy[:, :], vsl, E2[:, ei, :],
                                start=(j == 0), stop=last,
                                skip_group_check=True)
                # readout: yu + den per head (psY recycles as early as
                # possible), then the shared reciprocal chain. For the LAST
                # pair there is no next pair waiting on psY, so the
                # denominators go first and the tail psb matmul waits ~1.3us
                # less.
                yus = {}
                den2 = small.tile([33, GW], f32, tag="den2", name=f"den2_{g}_{pr}")
                den_first = (g, pr) == (1, 5)
                if den_first:
                    # whole reciprocal chain ahead of the y copies in the
                    # DVE FIFO: the tail psb matmul waits ~1.3us less
                    nc.vector.tensor_copy(den2[0:1, :], psyA[64:65, :])
                    nc.vector.tensor_copy(den2[32:33, :], psyB[64:65, :])
                else:
                    for i, (h, psy) in enumerate(((hA, psyA), (hB, psyB))):
                        yu = yup.tile([64, GW], f16, tag="yu", name="yu")
                        nc.vector.tensor_copy(yu[:, :], psy[0:64, :])
                        nc.vector.tensor_copy(den2[32 * i:32 * i + 1, :], psy[64:65, :])
                        yus[h] = yu
                rec2 = small.tile([33, GW], f32, tag="rec2", name=f"rec2_{g}_{pr}")
                nc.vector.reciprocal_approx_fast(out=rec2[:, :], in_=den2[:, :])
                rec16 = small.tile([33, GW], f16, tag="rec16", name=f"rec16_{g}_{pr}")
                nc.vector.tensor_copy(rec16[:, :], rec2[:, :])
                rec1s[(g, 2 * pr)] = rec16
                rec1s[(g, 2 * pr + 1)] = rec16
                if den_first:
                    for i, (h, psy) in enumerate(((hA, psyA), (hB, psyB))):
                        yu = yup.tile([64, GW], f16, tag="yu", name="yu")
                        nc.vector.tensor_copy(yu[:, :], psy[0:64, :])
                        yus[h] = yu
                return yus

            def emit_norm_apply(g, pr, yus):
                for i in range(2):
                    h = 2 * pr + i
                    qt, qp = h // 2, (h % 2) * 64
                    psb = psG.tile([64, GW], f32, tag="g", name="psb")
                    nc.tensor.matmul(
                        psb[:, :], ones65[32 * i:32 * i + 1, :],
                        rec1s[(g, h)][32 * i:32 * i + 1, :], start=True, stop=True)
                    nc.vector.tensor_mul(
                        yT[qt][qp:qp + 64, g * GW:(g + 1) * GW],
                        yus[h][:, :], psb[:, :])

            # ---- group-0 prelude: first qk GEMMs, bias tiles, v tiles ----
            yus_all = {0: {}, 1: {}}
            emit_qk(0, 0)
            emit_qk(6, 0)

            # bias f16 casts + broadcast tiles (bqv32/bp32 landed long ago)
            bqv16 = consts.tile([1, C], f16, tag="bqv16")
            nc.vector.tensor_copy(bqv16[:, :], bqv32[:, :])
            bp16 = consts.tile([1, C], f16, tag="bp16")
            nc.vector.tensor_copy(bp16[:, :], bp32[:, :])
            vb_ps = psG.tile([128, 512], f32, tag="g", name="vb_ps")
            pbias = consts.tile([128, C], f32, tag="pbias")
            vbias = consts.tile([128, C], f16, tag="vbias")
            for c0 in (0, 512):
                w = min(512, C - c0)
                nc.tensor.matmul(
                    vb_ps[:, 0:w], ones16[0:1, 0:128],
                    bqv16[0:1, c0:c0 + w], start=True, stop=True)
                nc.vector.tensor_copy(vbias[:, c0:c0 + w], vb_ps[:, 0:w])
            for c0 in (0, 512):
                w = min(512, C - c0)
                pb_ps = psG.tile([128, 512], f32, tag="g", name="pb_ps")
                nc.tensor.matmul(
                    pb_ps[:, 0:w], ones16[0:1, 0:128],
                    bp16[0:1, c0:c0 + w], start=True, stop=True)
                nc.vector.tensor_copy(pbias[:, c0:c0 + w], pb_ps[:, 0:w])

            for mi, mv in enumerate(range(12, 18)):
                cast_w(mv, mi % 2 == 0)
            for t in range(4, MT):
                emit_x16(t)
            for t in range(4):
                emit_v(t)
            cast_w(1, True)
            cast_w(7, True)
            emit_qk(1, 0)
            emit_qk(7, 0)

            # ---- group-0 pair loop ----
            fill_cast = {0: (2, 8), 1: (3, 9), 2: (4, 10), 3: (5, 11)}
            fill_qk0 = {1: (2, 8), 2: (3, 9), 3: (4, 10), 4: (5, 11)}
            fill_v = {2: 4, 3: 5, 4: 6, 5: 7}
            fill_wp = {2: (0, 1, 2), 3: (3, 4, 5)}
            g0_fillers = {
                0: (lambda: emit_transpose(4)),
                1: (lambda: emit_transpose(5)),
                2: (lambda: emit_transpose(6)),
                3: (lambda: emit_qk(1, 1)),
                4: (lambda: emit_qk(2, 1)),
                5: (lambda: emit_qk(3, 1)),
            }
            fill_qk1post = {2: (0, 6), 3: (7,), 4: (8,), 5: (9,)}
            for pr in range(6):
                yus_all[0].update(emit_pair(0, pr, filler=g0_fillers[pr]))
                if pr == 2:
                    emit_transpose(7)
                for mi, m in enumerate(fill_cast.get(pr, ())):
                    cast_w(m, mi % 2 == 0)
                for m in fill_qk0.get(pr, ()):
                    emit_qk(m, 0, bias_dve=True)
                for m in fill_qk1post.get(pr, ()):
                    emit_qk(m, 1)
                if pr in fill_v:
                    emit_v(fill_v[pr])
                for k in fill_wp.get(pr, ()):
                    load_wp(k)
                # delay each norm apply by one pair: its psb matmul waits on
                # the DVE reciprocal, so give it PE-filler lead time.
                if pr >= 1:
                    emit_norm_apply(0, pr - 1, yus_all[0])
            for m in (4, 10, 5, 11):
                emit_qk(m, 1)
            emit_norm_apply(0, 5, yus_all[0])

            # ---- group-1 pair loop ----
            for pr in range(6):
                if pr < 4:
                    filler = lambda t=pr: emit_proj_ks(t, [0, 1, 2])
                elif pr == 4:
                    filler = lambda: emit_proj_partial(5, [0, 1, 2, 3])
                else:
                    filler = lambda: emit_proj_partial(6, [0, 1, 2, 3, 4])
                yus_all[1].update(emit_pair(1, pr, filler=filler))
                if pr < 4:
                    emit_proj_ks(pr, [3, 4, 5])
                    emit_proj_fin(pr)
                    emit_norm_apply(1, pr, yus_all[1])
                elif pr == 4:
                    emit_proj_partial(4, [0, 1, 2, 3])
                    emit_norm_apply(1, 4, yus_all[1])
            emit_proj_partial(7, [0, 1, 2, 3, 4])
            emit_norm_apply(1, 5, yus_all[1])
            emit_proj_final(4, [4, 5])
            emit_proj_final(5, [4, 5])
            emit_proj_final(6, [5])
            emit_proj_final(7, [5])

    nc.finalize()
    return nc


_CACHE = {}


def _get_nc():
    if "nc" not in _CACHE:
        _CACHE["nc"] = build_nc()
    return _CACHE["nc"]


def run(inputs, trace=False):
    nc = _get_nc()
    x = np.asarray(inputs["x"], dtype=np.float32)
    in_maps = [
        {
            "x": np.ascontiguousarray(x[i]),
            "W_attn": np.asarray(inputs["W_attn"], dtype=np.float32),
            "b_attn": np.asarray(inputs["b_attn"], dtype=np.float32),
            "W_proj": np.asarray(inputs["W_proj"], dtype=np.float32),
            "b_proj": np.asarray(inputs["b_proj"], dtype=np.float32),
        }
        for i in range(B)
    ]
    res = run_bass_kernel_spmd(nc, in_maps, core_ids=list(range(B)), trace=trace)
    y = np.stack([res.results[i]["out"] for i in range(B)], axis=0)
    return y, res


def kernel(**inputs):
    y, _ = run(inputs, trace=False)
    return y
